# revision 79
# baseline (speedup 1.0000x reference)
"""Trainium2 Bass kernel for nn_LookaheadModel (topk_masking).

Sharding: data-parallel over batch B=8 (one batch element per core) for the
encoder; tiny AllGather of per-batch context vectors; vocab-sharded output
projection (each core computes logits[:, shard]).

v2 layout: activations kept feature-major in the pre-layernorm basis y
(yT tiles: D on partitions, T free).  LN is folded into per-token row
scalars r_t = rsqrt(var+eps), mr_t = mean*r: h = r*(g.*y) - mr*g + b.
Gate / future-phi / score projections are fused into one 3-row matmul per
chunk against host-premultiplied (gate_W .* ln_g) vectors, with row-space
corrections.  All top-k selection and softmax math runs in a column-major
(128 x 32) layout so vector ops use all 128 lanes.  out_W (f16) is fully
prefetched during the encoder so the final projection is PE-bound.

Self-contained: only needs numpy + the system-installed concourse package.
"""

import numpy as np

import bass_rust
import concourse.bass as bass
import concourse.mybir as mybir
from concourse.bass_utils import run_bass_kernel_spmd
from concourse.tile import TileContext

AF = mybir.ActivationFunctionType
ALU = mybir.AluOpType
F32 = mybir.dt.float32
F32R = mybir.dt.float32r
F16 = mybir.dt.float16
BF16 = mybir.dt.bfloat16
I32 = mybir.dt.int32
U8 = mybir.dt.uint8

# ---------------------------------------------------------------------------
# Workaround: this walrus build rejects any instruction carrying more than one
# sync-wait command. Hoist excess waits onto same-engine NOPs (sequential on
# the same engine queue, so semantically identical).
# ---------------------------------------------------------------------------
_MAX_WAITS = 1
_nop_counter = [0]


def _split_waits_in_ordered(nc, ordered):
    for bb_name, insts in ordered.items():
        out = []
        for inst in insts:
            si = inst.sync_info
            waits = list(si.on_wait) if si and si.on_wait else []
            if len(waits) > _MAX_WAITS:
                spill, keep = waits[:-_MAX_WAITS], waits[-_MAX_WAITS:]
                for i in range(0, len(spill), _MAX_WAITS):
                    _nop_counter[0] += 1
                    nop = bass_rust.InstNoOp(name=f"WSPILL-{_nop_counter[0]}")
                    nop.engine = inst.engine
                    nop.sync_info = mybir.SyncInfo(
                        on_wait=list(spill[i : i + _MAX_WAITS]), on_update=[]
                    )
                    nop.bass_nofuse = True
                    nc.register_instruction(nop, overwrite=True)
                    out.append(nop)
                si.on_wait = keep
            out.append(inst)
        if len(out) != len(insts):
            insts[:] = out


_orig_lower = TileContext._lower_ordered_insts
_orig_drain = TileContext._drain_and_barrier


def _lower_with_split(self, ordered):
    _split_waits_in_ordered(self.nc, ordered)
    return _orig_lower(self, ordered)


def _drain_and_barrier_split(self, tick_clock, wait_clock):
    nc = self.nc
    sc = bass_rust.ScopedClock({None: tick_clock.global_clock})
    drain_inst = nc.sync.drain()
    wait_clock.add_sem_waits(drain_inst.ins, sc)
    si = drain_inst.ins.sync_info
    waits = list(si.on_wait or [])
    if len(waits) > _MAX_WAITS:
        si.on_wait = waits[:_MAX_WAITS]
        rest = waits[_MAX_WAITS:]
        for i in range(0, len(rest), _MAX_WAITS):
            nop = nc.sync.nop(nofuse=True, hint=f"drain_wait_spill_{i}")
            nop.ins.sync_info = mybir.SyncInfo(
                on_wait=list(rest[i : i + _MAX_WAITS]), on_update=[]
            )
    nc.all_engine_barrier()
    popped = nc._tile_sem_poison_stack.pop()
    assert popped is self._sem_poison
    nc.clear_and_free_semaphores(list(self.sems.allocated().values()))
    nc.all_engine_barrier()


def _apply_patch():
    TileContext._drain_and_barrier = _drain_and_barrier_split
    TileContext._lower_ordered_insts = _lower_with_split


# ---------------------------------------------------------------------------
# Problem constants
# ---------------------------------------------------------------------------
V, D, SLOTS, K = 50257, 512, 256, 8
B, T = 8, 4096
NCORES = 8
VS = 6283  # vocab shard width per core; 8*6283 = 50264 >= V
NCH = 8  # T chunks of width 512
CW = 512
NK = D // 128  # 4 feature tiles
NF = 2 * D // 128  # 8 hidden tiles
TC = T // 128  # 32 col-layout width
BIG = 1.0e30
EPS = 1e-5

PURE_FP32 = False  # kept for test.py compat (ignored; kernel is f32r/f16)
USE_PBCAST = False  # partition_broadcast unsupported by this walrus codegen
USE_GP_CTXY = False  # Pool engine lacks TensorScalarPtr
DEBUG_HT = False  # adds dbg row dumps (bring-up only)

_cache = {}


def build_bass():
    _apply_patch()
    DT = F16
    FDT = F16
    nc = bass.Bass(trn_type="TRN2", num_devices=NCORES)

    # ---- I/O ----
    embed = nc.dram_tensor("embed", (V, D), F16, kind="ExternalInput")
    seq_idx = nc.dram_tensor("seq_idx", (128, TC), I32, kind="ExternalInput")
    w1 = nc.dram_tensor("w1", (D, 2 * D), F16, kind="ExternalInput")
    w2 = nc.dram_tensor("w2", (2 * D, D), F16, kind="ExternalInput")
    qw = nc.dram_tensor("qw", (D, D), F16, kind="ExternalInput")
    # packed small constants; see cpk layout at the load site
    cpack_in = nc.dram_tensor("cpack", (128, 84), F32, kind="ExternalInput")
    idsh_in = nc.dram_tensor("idsh", (128, 256), F32, kind="ExternalInput")
    NCHUNKS = (VS + CW - 1) // CW
    wout = nc.dram_tensor("wout", (128, NCHUNKS * NK * CW), FDT, kind="ExternalInput")
    boutr = nc.dram_tensor("boutr", (B, VS), F32, kind="ExternalInput")

    logits = nc.dram_tensor("logits", (B, VS), F32, kind="ExternalOutput")
    dbg = nc.dram_tensor("dbg", (8, T), F32, kind="ExternalOutput")

    cntrow_in = nc.dram_tensor("cntrow", (1, T), F16, kind="ExternalInput")
    rows_d = nc.dram_tensor("rows_d", (3, T), F32, kind="Internal")
    zrow_d = nc.dram_tensor("zrow_d", (1, T), F32, kind="Internal")
    wrow_d = nc.dram_tensor("wrow_d", (1, T), F32, kind="Internal")
    wrowh_d = nc.dram_tensor("wrowh_d", (1, T), BF16, kind="Internal")
    cc_din = nc.dram_tensor("cc_din", (128, 1), F32, kind="Internal")
    cc_dout = nc.dram_tensor(
        "cc_dout", (128 * NCORES, 1), F32, kind="Internal", addr_space="Shared"
    )
    cc_in = nc.dram_tensor("cc_in", (128, NK), F32, kind="Internal")
    cc_out = nc.dram_tensor(
        "cc_out", (128 * NCORES, NK), F32, kind="Internal", addr_space="Shared"
    )

    def col2row(drow, ctile):
        # SBUF col tile (128, TC) -> DRAM row (1, T), t = TC*p + c
        return drow[:].rearrange("o (p c) -> (o p) c", p=128), ctile[:]

    with TileContext(nc) as tc:
        with tc.tile_pool(name="consts", bufs=1) as cpool:
            # ---------------- persistent constants ----------------
            # sidx first: the first gather (critical path) depends only on it
            sidx = cpool.tile([128, TC], I32, name="sidx")
            nc.sync.dma_start(sidx[:], seq_idx[:])
            # all small constants packed into one DMA (cpack (128, 84)):
            # [b1 0:8 | b2 8:12 | g4 12:16 | b4 16:20 | qb4 20:24 | sgw 24:40 |
            #  alpha 40 | cntrec 41:73 | row0 73:81 = scal]
            cpk = cpool.tile([128, 84], F32, name="cpk")
            nc.sync.dma_start(cpk[:], cpack_in[:])
            b1t = cpk[:, 0:8]
            b2t = cpk[:, 8:12]
            g4c = cpk[:, 12:16]
            b4c = cpk[:, 16:20]
            qb4c = cpk[:, 20:24]
            cntc = cpk[:, 41:73]
            alphac = cpk[:, 40:41]
            scin = cpk[0:1, 73:81]
            eps_ap = cpk[0:1, 73:74]
            idsh = cpool.tile([128, 256], F32, name="idsh")
            nc.sync.dma_start(idsh[:], idsh_in[:])
            ident = idsh[:, 0:128]
            sht = idsh[:, 128:256]
            # fused per-k-tile lhsT: [g1g, g2g, qg(later), ones] per k
            sgw = cpool.tile([128, 4 * NK], DT, name="sgw")
            nc.vector.tensor_copy(sgw[:], cpk[:, 24:40])
            onescol = cpool.tile([128, 1], F32, name="onescol")
            nc.vector.memset(onescol[:], 1.0)
            identr = cpool.tile([128, 128], DT, name="identr")
            nc.vector.tensor_copy(identr[:], ident)
            onescol_r = cpool.tile([128, 1], F32R, name="onescol_r")
            nc.vector.tensor_copy(onescol_r[:], onescol[:])
            ones1x128 = cpool.tile([1, 128], F32, name="ones1x128")
            nc.vector.memset(ones1x128[:], 1.0)
            ones1x128h = cpool.tile([1, 128], BF16, name="ones1x128h")
            nc.vector.memset(ones1x128h[:], 1.0)
            strip = cpool.tile([1, 64], F32, name="strip")
            nc.vector.memset(strip[0:1, 40:41], BIG)
            nc.vector.memset(strip[0:1, 41:42], -BIG)
            q4 = cpool.tile([128, NK], F32, name="q4")
            hl = cpool.tile([128, NK], DT, name="hl")
            ctxY = cpool.tile([128, NK], F32, name="ctxY")
            ctx4 = cpool.tile([128, NK], F32, name="ctx4")
            ctxall = cpool.tile([128, 4 * NCORES], F32, name="ctxall")

            nchunks = (VS + CW - 1) // CW
            NWO_BUF = 13  # all out_W chunks resident (full prefetch)
            with tc.tile_pool(name="wo", bufs=NWO_BUF) as wopool:
                # out_W streamed in chunk-sized single DMAs (f16); the 4 k-tile
                # blocks of a chunk land side by side in one [128, 4w] tile
                wotiles = []

                def load_wochunk(n, eng=None):
                    w = min(CW, VS - n * CW)
                    wt = wopool.tile([128, NK * CW], FDT, name=f"wo{n}", tag="wo")
                    (eng or nc.sync).dma_start(
                        wt[:, : NK * w],
                        wout[:, n * NK * CW : n * NK * CW + NK * w],
                    )
                    wotiles.append((wt, w))

                with tc.tile_pool(name="yT", bufs=1) as hpool:
                    yT = [hpool.tile([128, T], DT, name=f"yT{k}") for k in range(NK)]
                    # row-space pipeline state lives in its own pool, closed
                    # before the projection so its SBUF is reusable
                    _rowcm = tc.tile_pool(name="rows", bufs=1)
                    rowpool = _rowcm.__enter__()
                    phirow = rowpool.tile([1, T + 8], F32, name="phirow")
                    nc.vector.memset(phirow[0:1, T : T + 8], 0.0)
                    zrowA = rowpool.tile([1, T], F32, name="zrowA")
                    cntrow = rowpool.tile([1, T], F16, name="cntrow")
                    nc.sync.dma_start(cntrow[:], cntrow_in[:])

                    # ---------------- phase A ----------------
                    with (
                        tc.tile_pool(name="wts", bufs=1) as wpool,
                        tc.tile_pool(name="gat", bufs=4) as gpool,
                        tc.tile_pool(name="x0p", bufs=2) as x0pool,
                        tc.tile_pool(name="ap", bufs=1) as apool,
                        tc.tile_pool(name="sqp", bufs=2) as sqpool,
                        tc.tile_pool(name="stp", bufs=2) as stpool,
                        tc.tile_pool(name="st1", bufs=1) as st1pool,
                        tc.tile_pool(name="pstp", bufs=2, space="PSUM") as pstp,
                        tc.tile_pool(name="psa", bufs=2, space="PSUM") as psa,
                        tc.tile_pool(name="psf", bufs=2, space="PSUM") as psf,
                        tc.tile_pool(name="prow", bufs=1, space="PSUM") as prows,
                        tc.tile_pool(name="paux", bufs=1, space="PSUM") as paux,
                    ):
                        # f16 weights: DMA straight into matmul operand tiles
                        w1t, w2t, qwt = [], [], []
                        for k in range(NK):
                            wr = wpool.tile([128, 2 * D], DT, name=f"w1r{k}")
                            nc.sync.dma_start(wr[:], w1[128 * k : 128 * (k + 1), :])
                            w1t.append(wr)
                        for k in range(NF):
                            wr = wpool.tile([128, D], DT, name=f"w2r{k}")
                            nc.sync.dma_start(wr[:], w2[128 * k : 128 * (k + 1), :])
                            w2t.append(wr)
                        for k in range(NK):
                            wr = wpool.tile([128, D], DT, name=f"qwr{k}")
                            nc.sync.dma_start(wr[:], qw[128 * k : 128 * (k + 1), :])
                            qwt.append(wr)
                        # warm the collective stream early: a dummy
                        # AllGather absorbs the fixed ~11.5us trigger-to-start
                        # cost during the encoder instead of the real CC
                        nc.gpsimd.collective_compute(
                            "AllGather",
                            ALU.bypass,
                            replica_groups=[list(range(NCORES))],
                            ins=[cc_din[:]],
                            outs=[cc_dout[:]],
                        )
                        # out_W chunk 0 now; the rest stream 2-per-chunk from
                        # inside the loop so no queue jams at startup
                        load_wochunk(0, nc.sync)
                        wo_next = [1]

                        aux = paux.tile([128, CW], F32, name="aux")

                        order = [7] + list(range(7))
                        for idx, ch in enumerate(order):
                            sl = slice(ch * CW, (ch + 1) * CW)
                            x0 = [
                                x0pool.tile([128, CW], DT, name=f"x0_{k}", tag=f"x0_{k}")
                                for k in range(NK)
                            ]
                            for blk in range(4):
                                tb = 4 * ch + blk
                                g = gpool.tile([128, D], DT, name="g", tag="g")
                                nc.gpsimd.indirect_dma_start(
                                    out=g[:],
                                    out_offset=None,
                                    in_=embed[:],
                                    in_offset=bass.IndirectOffsetOnAxis(
                                        ap=sidx[:, tb : tb + 1], axis=0
                                    ),
                                )
                                tp = pstp.tile([128, D], DT, tag="tp")
                                for k in range(NK):
                                    nc.tensor.transpose(
                                        tp[:, 128 * k : 128 * (k + 1)],
                                        g[:, 128 * k : 128 * (k + 1)],
                                        identr,
                                    )
                                for k in range(NK):
                                    if k % 2 == 0:
                                        nc.vector.tensor_copy(
                                            x0[k][:, 128 * blk : 128 * (blk + 1)],
                                            tp[:, 128 * k : 128 * (k + 1)],
                                        )
                                    else:
                                        nc.scalar.activation(
                                            x0[k][:, 128 * blk : 128 * (blk + 1)],
                                            tp[:, 128 * k : 128 * (k + 1)],
                                            AF.Copy,
                                        )
                            # layer 1 + relu
                            af = [
                                apool.tile([128, CW], DT, name=f"af{m}", tag=f"af{m}")
                                for m in range(NF)
                            ]
                            for m in range(NF):
                                ps = psa.tile([128, CW], F32, tag="psa")
                                for k in range(NK):
                                    nc.tensor.matmul(
                                        ps[:],
                                        lhsT=w1t[k][:, 128 * m : 128 * (m + 1)],
                                        rhs=x0[k][:],
                                        start=(k == 0),
                                        stop=(k == NK - 1),
                                    )
                                nc.scalar.activation(
                                    af[m][:], ps[:], AF.Relu, bias=b1t[:, m : m + 1]
                                )
                            # layer 2 + bias + residual -> yT directly
                            for m in range(NK):
                                ps = psf.tile([128, CW], F32, tag="psf")
                                for k in range(NF):
                                    nc.tensor.matmul(
                                        ps[:],
                                        lhsT=w2t[k][:, 128 * m : 128 * (m + 1)],
                                        rhs=af[k][:],
                                        start=(k == 0),
                                        stop=(k == NF - 1),
                                    )
                                nc.vector.scalar_tensor_tensor(
                                    out=yT[m][:, sl],
                                    in0=ps[:],
                                    scalar=b2t[:, m : m + 1],
                                    in1=x0[m][:],
                                    op0=ALU.add,
                                    op1=ALU.add,
                                )
                            # fused rows matmul: [sum(y), s1, s2, sq] in one
                            # 4-row group per k-tile (sq row needs qg — garbage
                            # for the first chunk (7), patched after q below)
                            pr = prows.tile([128, CW], F32, tag="rows")
                            for k in range(NK):
                                nc.tensor.matmul(
                                    pr[0:4, :],
                                    lhsT=sgw[:, 4 * k : 4 * k + 4],
                                    rhs=yT[k][:, sl],
                                    start=(k == 0),
                                    stop=(k == NK - 1),
                                )
                            nrows = 3 if idx == 0 else 4
                            stg = stpool.tile([4, CW], F32, name="stg", tag="stg")
                            nc.vector.tensor_copy(stg[0:nrows, :], pr[0:nrows, :])
                            if idx > 0:
                                nc.sync.dma_start(rows_d[0:1, sl], stg[3:4, :])
                            # s1/s2 rows to partition 0 (engines can't read
                            # SBUF at partition offsets 1-2; DMAs can)
                            s12 = stpool.tile([1, 2 * CW], F32, name="s12", tag="s12")
                            nc.sync.dma_start(s12[0:1, :], stg[1:3, :])
                            for m in range(NK):
                                sq = sqpool.tile([128, CW], F32R, name="sq", tag="sq")
                                nc.scalar.activation(sq[:], yT[m][:, sl], AF.Square)
                                nc.tensor.matmul(
                                    aux[0:1, :],
                                    lhsT=onescol_r[:],
                                    rhs=sq[:],
                                    start=(m == 0),
                                    stop=(m == NK - 1),
                                )
                            # row chain: r = rsqrt(var+eps) via Ln/Exp; mr = m*r
                            st = st1pool.tile([1, 9 * CW + 32], F32, name="st", tag="st")
                            mrow = st[0:1, 0:CW]
                            ex2 = st[0:1, CW : 2 * CW]
                            t1r = st[0:1, 2 * CW : 3 * CW]
                            r_sl = st[0:1, 3 * CW : 4 * CW]
                            mr_sl = st[0:1, 4 * CW : 5 * CW]
                            nc.vector.tensor_scalar(
                                out=mrow, in0=pr[0:1, :], scalar1=1.0 / D,
                                scalar2=None, op0=ALU.mult,
                            )
                            nc.vector.tensor_scalar(
                                out=ex2, in0=aux[0:1, :], scalar1=1.0 / D,
                                scalar2=None, op0=ALU.mult,
                            )
                            nc.vector.tensor_mul(t1r, mrow, mrow)
                            nc.vector.tensor_sub(ex2, ex2, t1r)
                            nc.scalar.activation(t1r, ex2, AF.Ln, bias=eps_ap)
                            nc.scalar.activation(r_sl, t1r, AF.Exp, scale=-0.5)
                            nc.vector.tensor_mul(mr_sl, mrow, r_sl)
                            nc.sync.dma_start(rows_d[1:2, sl], r_sl)
                            nc.sync.dma_start(rows_d[2:3, sl], mr_sl)
                            for _ in range(2):
                                if wo_next[0] < nchunks:
                                    load_wochunk(wo_next[0], nc.sync)
                                    wo_next[0] += 1

                            # row-space gate pipeline: phi and the z base for
                            # this chunk; finalize z of the chunk whose future
                            # window is now complete
                            ptmp = st[0:1, 5 * CW : 6 * CW]
                            nc.vector.tensor_mul(ptmp, r_sl, s12[0:1, CW : 2 * CW])
                            nc.vector.scalar_tensor_tensor(
                                out=ptmp, in0=mr_sl, scalar=cpk[0:1, 76:77],
                                in1=ptmp, op0=ALU.mult, op1=ALU.add,
                            )
                            nc.scalar.activation(
                                phirow[0:1, sl], ptmp, AF.Identity,
                                bias=cpk[0:1, 77:78],
                            )
                            nc.vector.tensor_mul(ptmp, r_sl, s12[0:1, 0:CW])
                            nc.vector.scalar_tensor_tensor(
                                out=ptmp, in0=mr_sl, scalar=cpk[0:1, 74:75],
                                in1=ptmp, op0=ALU.mult, op1=ALU.add,
                            )
                            nc.scalar.activation(
                                zrowA[0:1, sl], ptmp, AF.Identity,
                                bias=cpk[0:1, 75:76],
                            )

                            def fin_z(c):
                                s = c * CW
                                arow = st[0:1, 6 * CW : 6 * CW + 518]
                                brow = st[0:1, 7 * CW + 8 : 7 * CW + 524]
                                wrow = st[0:1, 8 * CW + 16 : 8 * CW + 528]
                                nc.vector.tensor_add(
                                    arow,
                                    phirow[0:1, s + 1 : s + 519],
                                    phirow[0:1, s + 2 : s + 520],
                                )
                                nc.vector.tensor_add(
                                    brow,
                                    st[0:1, 6 * CW : 6 * CW + 516],
                                    st[0:1, 6 * CW + 2 : 6 * CW + 518],
                                )
                                nc.vector.tensor_add(
                                    wrow,
                                    st[0:1, 7 * CW + 8 : 7 * CW + 520],
                                    st[0:1, 7 * CW + 12 : 7 * CW + 524],
                                )
                                nc.vector.tensor_mul(
                                    wrow, wrow, cntrow[0:1, s : s + CW]
                                )
                                nc.vector.tensor_add(
                                    zrowA[0:1, s : s + CW],
                                    zrowA[0:1, s : s + CW],
                                    wrow,
                                )
                                # running bounds for the bisection
                                zmn = st[0:1, 6 * CW : 6 * CW + 1]
                                zmx = st[0:1, 6 * CW + 1 : 6 * CW + 2]
                                nc.vector.tensor_reduce(
                                    out=zmn, in_=zrowA[0:1, s : s + CW],
                                    axis=mybir.AxisListType.X, op=ALU.min,
                                )
                                nc.vector.reduce_max(
                                    out=zmx, in_=zrowA[0:1, s : s + CW],
                                    axis=mybir.AxisListType.X,
                                )
                                nc.vector.tensor_tensor(
                                    out=strip[0:1, 40:41], in0=strip[0:1, 40:41],
                                    in1=zmn, op=ALU.min,
                                )
                                nc.vector.tensor_tensor(
                                    out=strip[0:1, 41:42], in0=strip[0:1, 41:42],
                                    in1=zmx, op=ALU.max,
                                )

                            if idx == 0:
                                fin_z(7)
                            elif idx >= 2:
                                fin_z(ch - 1)
                            if idx == 7:
                                fin_z(6)

                            if idx == 0:
                                # ---- q vector from the last token (chunk 7) ----
                                # bcast last-token r/mr (partition 0 slices)
                                nc.tensor.matmul(
                                    aux[:, 8:9], lhsT=ones1x128[:],
                                    rhs=r_sl[0:1, CW - 1 : CW], start=True, stop=True,
                                )
                                nc.tensor.matmul(
                                    aux[:, 9:10], lhsT=ones1x128[:],
                                    rhs=mr_sl[0:1, CW - 1 : CW], start=True, stop=True,
                                )
                                rlB = cpool.tile([128, 2], F32, name="rlB")
                                nc.vector.tensor_copy(rlB[:], aux[:, 8:10])
                                ylast = cpool.tile([128, NK], F32, name="ylast")
                                for k in range(NK):
                                    nc.vector.tensor_copy(
                                        ylast[:, k : k + 1],
                                        yT[k][:, T - 1 : T],
                                    )
                                # hl = (ylast*r - mr) * g + b
                                nc.vector.tensor_scalar(
                                    out=ylast[:], in0=ylast[:],
                                    scalar1=rlB[:, 0:1], scalar2=None, op0=ALU.mult,
                                )
                                nc.vector.tensor_scalar(
                                    out=ylast[:], in0=ylast[:],
                                    scalar1=rlB[:, 1:2], scalar2=None,
                                    op0=ALU.subtract,
                                )
                                nc.vector.tensor_mul(ylast[:], ylast[:], g4c[:])
                                nc.vector.tensor_add(hl[:], ylast[:], b4c[:])
                                # q row = hl^T @ qW + qb
                                for k in range(NK):
                                    nc.tensor.matmul(
                                        aux[0:1, :],
                                        lhsT=hl[:, k : k + 1],
                                        rhs=qwt[k][:],
                                        start=(k == 0),
                                        stop=(k == NK - 1),
                                    )
                                qrow = cpool.tile([1, D], F32, name="qrow")
                                nc.vector.tensor_copy(qrow[:], aux[0:1, :])
                                # transpose q row -> q4 cols; add qb in col form
                                for k in range(NK):
                                    nc.tensor.transpose(
                                        aux[:, 16 + k : 17 + k],
                                        qrow[0:1, 128 * k : 128 * (k + 1)],
                                        ident[0:1, 0:1],
                                    )
                                nc.vector.tensor_add(
                                    q4[:], aux[:, 16 : 16 + NK], qb4c[:]
                                )
                                # qg into sgw cols 4k+3
                                for k in range(NK):
                                    nc.vector.tensor_mul(
                                        sgw[:, 4 * k + 3 : 4 * k + 4],
                                        q4[:, k : k + 1],
                                        g4c[:, k : k + 1],
                                    )
                                # Aq = sum(q*g), Bq = sum(q*b)
                                qgb = cpool.tile([128, 2 * NK], F32, name="qgb")
                                nc.vector.tensor_mul(qgb[:, 0:NK], q4[:], g4c[:])
                                nc.vector.tensor_mul(qgb[:, NK : 2 * NK], q4[:], b4c[:])
                                nc.tensor.matmul(
                                    aux[0:1, 32 : 32 + 2 * NK],
                                    lhsT=onescol[:],
                                    rhs=qgb[:],
                                    start=True,
                                    stop=True,
                                )
                                # strip[0,0]=Aq, strip[0,1]=Bq
                                nc.vector.tensor_reduce(
                                    out=strip[0:1, 0:1],
                                    in_=aux[0:1, 32 : 32 + NK],
                                    axis=mybir.AxisListType.X,
                                    op=ALU.add,
                                )
                                nc.vector.tensor_reduce(
                                    out=strip[0:1, 1:2],
                                    in_=aux[0:1, 32 + NK : 32 + 2 * NK],
                                    axis=mybir.AxisListType.X,
                                    op=ALU.add,
                                )
                                # deferred sq row for chunk 7 (q now known)
                                sl7 = slice(7 * CW, 8 * CW)
                                prd = prows.tile([128, CW], F32, tag="rows")
                                for k in range(NK):
                                    nc.tensor.matmul(
                                        prd[0:1, :],
                                        lhsT=sgw[:, 4 * k + 3 : 4 * k + 4],
                                        rhs=yT[k][:, sl7],
                                        start=(k == 0),
                                        stop=(k == NK - 1),
                                    )
                                stg7 = stpool.tile(
                                    [4, CW], F32, name="stg", tag="stg"
                                )
                                nc.vector.tensor_copy(stg7[0:1, :], prd[0:1, :])
                                nc.sync.dma_start(rows_d[0:1, sl7], stg7[0:1, :])

                    # ---------------- phase B: col-space selection ----------------
                    with (
                        tc.tile_pool(name="colp", bufs=1) as colp,
                        tc.tile_pool(name="bigp", bufs=1) as bigp,
                        tc.tile_pool(name="psm", bufs=1, space="PSUM") as psm,
                        tc.tile_pool(name="pwd", bufs=2, space="PSUM") as pwd,
                    ):
                        TH = T // 2
                        sm = psm.tile([128, CW], F32, name="sm")
                        zB = bigp.tile([128, T], F32, name="zB")
                        scr = bigp.tile([128, TH], F16, name="scr")
                        scrB = bigp.tile([128, TH], F16, name="scrB")
                        wB = bigp.tile([128, T], BF16, name="wB")
                        scrh = bigp.tile([128, T], F16, name="scrh")

                        # sq/r/mr rows -> col layout in ONE DMA; z comes from
                        # the row-space pipeline (zrowA) built during phase A
                        colpk3 = colp.tile([128, 3 * TC], F32, name="colpk3")
                        nc.sync.dma_start(
                            colpk3[:].rearrange("p (i c) -> p i c", i=3),
                            rows_d[:, :].rearrange("i (p c) -> p i c", p=128),
                        )
                        sqc = colpk3[:, 0 * TC : 1 * TC]
                        rc = colpk3[:, 1 * TC : 2 * TC]
                        mrc = colpk3[:, 2 * TC : 3 * TC]
                        sA = colp.tile([128, 40], F32, name="sA")
                        zc = colp.tile([128, TC], F32, name="zc")
                        uc = colp.tile([128, TC], F32, name="uc")
                        tc_ = colp.tile([128, TC], F32, name="tc_")
                        mq = colp.tile([128, TC], F32, name="mq")
                        gtv = colp.tile([128, TC], F32, name="gtv")
                        selc = colp.tile([128, TC], F32, name="selc")
                        ec = colp.tile([128, TC], F32, name="ec")
                        wcol = colp.tile([128, TC], BF16, name="wcol")
                        mask_u8 = colp.tile([128, TC], U8, name="mask_u8")
                        coltmp = colp.tile([128, 16], F32, name="coltmp")
                        zrow_sb = colp.tile([1, T], F32, name="zrow_sb")
                        wrow_sb = colp.tile([1, T], BF16, name="wrow_sb")
                        rsc = colp.tile([1, 256], F32, name="rsc")

                        def bcast(src11, dst_col):
                            # (1,1) -> (128,1) via PE
                            p = sm[:, 12:13]
                            nc.tensor.matmul(
                                p, lhsT=ones1x128[:], rhs=src11, start=True, stop=True
                            )
                            nc.vector.tensor_copy(dst_col, p)

                        # z row -> col tile via DRAM bounce (off critical path:
                        # zc is only needed after the bisection rounds)
                        nc.sync.dma_start(zrow_d[0:1, :], zrowA[0:1, :])
                        nc.sync.dma_start(
                            zc[:], zrow_d[:].rearrange("o (p c) -> (o p) c", p=128)
                        )
                        # replicate z across partitions straight from zrowA
                        for chx in range(NCH):
                            slx = slice(chx * CW, (chx + 1) * CW)
                            pb = pwd.tile([128, CW], F32, tag="pb")
                            nc.tensor.matmul(
                                pb[:], lhsT=ones1x128[:],
                                rhs=zrowA[0:1, slx], start=True, stop=True,
                            )
                            if chx % 2 == 0:
                                nc.vector.tensor_copy(zB[:, slx], pb[:])
                            else:
                                nc.scalar.activation(zB[:, slx], pb[:], AF.Copy)

                        # lo/hi bounds: accumulated in row space during phase A
                        lo0 = strip[0:1, 40:41]
                        hi0 = strip[0:1, 41:42]

                        N_ROUNDS = 3
                        lo_cur, hi_cur = lo0, hi0
                        si = 6
                        tau_col = coltmp[:, 6:7]
                        dB = coltmp[:, 7:8]
                        loB = coltmp[:, 8:9]
                        cnt_col = coltmp[:, 9:10]
                        cnt_col2 = coltmp[:, 12:13]
                        sgn_col = coltmp[:, 10:11]
                        for r in range(N_ROUNDS):
                            # pack [dd, lo] adjacently, one bcast matmul for both
                            dd = strip[0:1, si : si + 1]
                            nc.vector.tensor_scalar(
                                out=dd, in0=hi_cur, scalar1=lo_cur, scalar2=1.0 / 128,
                                op0=ALU.subtract, op1=ALU.mult,
                            )
                            nc.scalar.activation(
                                strip[0:1, si + 1 : si + 2], lo_cur, AF.Copy
                            )
                            p2 = sm[:, 14:16]
                            nc.tensor.matmul(
                                p2, lhsT=ones1x128[:], rhs=strip[0:1, si : si + 2],
                                start=True, stop=True,
                            )
                            nc.scalar.activation(dB, p2[:, 0:1], AF.Copy)
                            nc.scalar.activation(loB, p2[:, 1:2], AF.Copy)
                            # tau = alpha * (128*dd) + lo == alpha*d + lo
                            nc.vector.scalar_tensor_tensor(
                                out=tau_col, in0=alphac, scalar=dB, in1=loB,
                                op0=ALU.mult, op1=ALU.add,
                            )
                            ntau_col = sA[:, 20:21]
                            nc.vector.tensor_scalar(
                                out=ntau_col, in0=tau_col, scalar1=-1.0,
                                scalar2=None, op0=ALU.mult,
                            )
                            # count split: DVE is_gt on the first half, ACT
                            # Sign on the second half (count = (sgn+TH)/2)
                            sgnB_col = sA[:, 21:22]
                            nc.scalar.activation(
                                scrB[:], zB[:, TH:T], AF.Sign, bias=ntau_col,
                                accum_out=sgnB_col,
                            )
                            nc.vector.scalar_tensor_tensor(
                                out=scr[:],
                                in0=zB[:, 0:TH],
                                scalar=tau_col,
                                in1=zB[:, 0:TH],
                                op0=ALU.is_gt,
                                op1=ALU.bypass,
                                accum_out=cnt_col,
                            )
                            # 2*cntA + sgnB >= 2*SLOTS - TH  <=>  count >= SLOTS
                            cnt2x = sA[:, 22:23]
                            nc.vector.scalar_tensor_tensor(
                                out=cnt2x, in0=cnt_col, scalar=2.0, in1=sgnB_col,
                                op0=ALU.mult, op1=ALU.add,
                            )
                            nc.vector.tensor_scalar(
                                out=sgn_col, in0=cnt2x,
                                scalar1=float(2 * SLOTS - TH),
                                scalar2=None, op0=ALU.is_ge,
                            )
                            pj = sm[0:1, 0:1]
                            nc.tensor.matmul(
                                pj, lhsT=sgn_col, rhs=onescol[:], start=True, stop=True
                            )
                            # lo_n = lo + pj*dd ; hi_n = lo_n + dd
                            lo_n = strip[0:1, si + 2 : si + 3]
                            nc.vector.scalar_tensor_tensor(
                                out=lo_n, in0=pj, scalar=dd, in1=lo_cur,
                                op0=ALU.mult, op1=ALU.add,
                            )
                            hi_n = strip[0:1, si + 3 : si + 4]
                            nc.vector.tensor_add(hi_n, lo_n, dd)
                            lo_cur, hi_cur = lo_n, hi_n
                            si += 4

                        # v0 = min(z > lo_cur) exactly (col space)
                        loB2 = coltmp[:, 11:12]
                        bcast(lo_cur, loB2)
                        nc.vector.tensor_scalar(
                            out=mask_u8[:], in0=zc[:], scalar1=loB2, scalar2=None,
                            op0=ALU.is_gt,
                        )
                        nc.vector.memset(wcol[:], BIG)
                        nc.vector.copy_predicated(wcol[:], mask_u8[:], zc[:])
                        wmin_c = coltmp[:, 12:13]
                        nc.vector.tensor_reduce(
                            out=wmin_c, in_=wcol[:], axis=mybir.AxisListType.X, op=ALU.min
                        )
                        nc.tensor.transpose(sm[0:1, 128:256], wmin_c, ident[:])
                        v0 = strip[0:1, si : si + 1]
                        nc.vector.tensor_reduce(
                            out=v0, in_=sm[0:1, 128:256], axis=mybir.AxisListType.X,
                            op=ALU.min,
                        )
                        # exactly one token sits in the final bisection
                        # window (window << min z-gap), so sel = (z>v0) + (z==v0)
                        vB = coltmp[:, 13:14]
                        bcast(v0, vB)
                        nc.vector.scalar_tensor_tensor(
                            out=gtv[:], in0=zc[:], scalar=vB, in1=zc[:],
                            op0=ALU.is_gt, op1=ALU.bypass,
                        )
                        nc.vector.tensor_scalar(
                            out=mq[:], in0=zc[:], scalar1=vB, scalar2=None,
                            op0=ALU.is_equal,
                        )
                        nc.vector.tensor_add(selc[:], gtv[:], mq[:])

                        # masked softmax over scores (col space)
                        # s = r*sq - Aq*mr + Bq
                        BqB = coltmp[:, 4:5]
                        nAqB = coltmp[:, 5:6]
                        negaq = strip[0:1, 2:3]
                        nc.vector.tensor_scalar(
                            out=negaq, in0=strip[0:1, 0:1], scalar1=-1.0, scalar2=None,
                            op0=ALU.mult,
                        )
                        nc.tensor.matmul(
                            sm[:, 14:16], lhsT=ones1x128[:],
                            rhs=strip[0:1, 1:3], start=True, stop=True,
                        )
                        nc.vector.tensor_copy(coltmp[:, 4:6], sm[:, 14:16])
                        nc.vector.tensor_mul(tc_[:], rc[:], sqc[:])
                        nc.vector.scalar_tensor_tensor(
                            out=tc_[:], in0=mrc[:], scalar=nAqB, in1=tc_[:],
                            op0=ALU.mult, op1=ALU.add,
                        )
                        nc.vector.tensor_scalar(
                            out=tc_[:], in0=tc_[:], scalar1=BqB, scalar2=None, op0=ALU.add
                        )
                        # masked = s + BIG*(sel-1); (sel-1) FIRST to avoid 1e30+s
                        nc.vector.tensor_scalar(
                            out=uc[:], in0=selc[:], scalar1=-1.0, scalar2=None,
                            op0=ALU.add,
                        )
                        nc.vector.scalar_tensor_tensor(
                            out=uc[:], in0=uc[:], scalar=BIG, in1=tc_[:],
                            op0=ALU.mult, op1=ALU.add,
                        )
                        smx_c = coltmp[:, 6:7]
                        nc.vector.reduce_max(
                            out=smx_c, in_=uc[:], axis=mybir.AxisListType.X
                        )
                        nc.tensor.transpose(sm[0:1, 128:256], smx_c, ident[:])
                        smax = strip[0:1, si + 3 : si + 4]
                        nc.vector.reduce_max(
                            out=smax, in_=sm[0:1, 128:256], axis=mybir.AxisListType.X
                        )
                        nsmax = strip[0:1, si + 4 : si + 5]
                        nc.vector.tensor_scalar(
                            out=nsmax, in0=smax, scalar1=-1.0, scalar2=None, op0=ALU.mult
                        )
                        nsB = coltmp[:, 7:8]
                        bcast(nsmax, nsB)
                        zs_col = coltmp[:, 8:9]
                        nc.scalar.activation(
                            ec[:], uc[:], AF.Exp, bias=nsB, accum_out=zs_col
                        )
                        pz = sm[0:1, 2:3]
                        nc.tensor.matmul(
                            pz, lhsT=zs_col, rhs=onescol[:], start=True, stop=True
                        )
                        rz = strip[0:1, si + 5 : si + 6]
                        nc.vector.reciprocal(out=rz, in_=pz)
                        if DEBUG_HT:
                            nc.sync.dma_start(*col2row(dbg[0:1, :], zc))
                            nc.sync.dma_start(*col2row(dbg[1:2, :], selc))
                            nc.sync.dma_start(*col2row(dbg[2:3, :], tc_))  # s
                            nc.sync.dma_start(*col2row(dbg[3:4, :], ec))
                            nc.sync.dma_start(*col2row(dbg[4:5, :], selc))
                        # w = e*r (unnormalized); S2u = sum(e*mr)
                        nc.vector.tensor_mul(wcol[:], ec[:], rc[:])
                        nc.vector.tensor_mul(tc_[:], ec[:], mrc[:])
                        s2p = coltmp[:, 9:10]
                        nc.vector.tensor_reduce(
                            out=s2p, in_=tc_[:], axis=mybir.AxisListType.X, op=ALU.add
                        )
                        ps2u = sm[0:1, 3:4]
                        nc.tensor.matmul(
                            ps2u, lhsT=s2p, rhs=onescol[:], start=True, stop=True
                        )

                        # w col -> row -> replicate into wB (bf16)
                        nc.sync.dma_start(*col2row(wrowh_d, wcol))
                        nc.sync.dma_start(wrow_sb[:], wrowh_d[:])
                        if USE_PBCAST:
                            nc.gpsimd.partition_broadcast(wB[:], wrow_sb[0:1, :])
                        else:
                            for chx in range(NCH):
                                slx = slice(chx * CW, (chx + 1) * CW)
                                pb = pwd.tile([128, CW], F32, tag="pb")
                                nc.tensor.matmul(
                                    pb[:], lhsT=ones1x128h[:],
                                    rhs=wrow_sb[0:1, slx], start=True, stop=True,
                                )
                                if chx % 2 == 0:
                                    nc.vector.tensor_copy(wB[:, slx], pb[:])
                                else:
                                    nc.scalar.activation(wB[:, slx], pb[:], AF.Copy)
                        # ctxY[:, k] = sum_t w_t * y_k[:, t]; token range split
                        # across DVE and GpSimd, combined at the end
                        ctxYb = colp.tile([128, NK], F32, name="ctxYb")
                        for half, (h0, h1) in enumerate(((0, TH), (TH, T))):
                            acc = ctxY if half == 0 else ctxYb
                            for k in range(NK):
                                nc.vector.scalar_tensor_tensor(
                                    out=scrh[:, h0:h1],
                                    in0=yT[k][:, h0:h1],
                                    scalar=1.0,
                                    in1=wB[:, h0:h1],
                                    op0=ALU.mult,
                                    op1=ALU.mult,
                                    accum_out=acc[:, k : k + 1],
                                )
                        nc.vector.tensor_add(ctxY[:], ctxY[:], ctxYb[:])
                        # ctx = g*ctxY*rz - (S2u*rz)*g + b
                        rzB = coltmp[:, 10:11]
                        s2rz = strip[0:1, si + 6 : si + 7]
                        nc.vector.tensor_mul(s2rz, ps2u, rz)
                        s2rzB = coltmp[:, 11:12]
                        nc.tensor.matmul(
                            sm[:, 14:16], lhsT=ones1x128[:],
                            rhs=strip[0:1, si + 5 : si + 7], start=True, stop=True,
                        )
                        nc.vector.tensor_copy(coltmp[:, 10:12], sm[:, 14:16])
                        nc.vector.tensor_scalar(
                            out=ctxY[:], in0=ctxY[:], scalar1=rzB, scalar2=None,
                            op0=ALU.mult,
                        )
                        nc.vector.tensor_mul(ctxY[:], ctxY[:], g4c[:])
                        # u = s2rz*g - b ; ctx4 = ctxY - u
                        nc.vector.scalar_tensor_tensor(
                            out=ctx4[:], in0=g4c[:], scalar=s2rzB, in1=b4c[:],
                            op0=ALU.mult, op1=ALU.subtract,
                        )
                        nc.vector.tensor_sub(ctx4[:], ctxY[:], ctx4[:])

                        nc.sync.dma_start(cc_in[:], ctx4[:])
                        nc.gpsimd.collective_compute(
                            "AllGather",
                            ALU.bypass,
                            replica_groups=[list(range(NCORES))],
                            ins=[cc_in[:]],
                            outs=[cc_out[:]],
                        )

                        # PE p-state warmup: a post-CC token DMA gates dummy
                        # matmuls so the PE clock ramps while ctxall lands
                        nc.gpsimd.dma_start(
                            out=yT[0][:, 0:1], in_=cc_out[0:128, 0:1]
                        )
                        for _ in range(8):
                            wm = pwd.tile([128, CW], F32, tag="pb")
                            nc.tensor.matmul(
                                wm[:], lhsT=identr[:], rhs=yT[0][:, 0:CW],
                                start=True, stop=True,
                            )

                    # phase-B + row pools closed; SBUF free for the projection
                    _rowcm.__exit__(None, None, None)

                    # ---------------- output projection ----------------
                    nc.sync.dma_start(
                        ctxall[:].rearrange("p (j b) -> p j b", j=NK),
                        cc_out[:].rearrange("(b p) j -> p j b", p=128),
                    )
                    ctxall_h = cpool.tile([128, 4 * NCORES], F16, name="ctxall_h")
                    nc.vector.tensor_copy(ctxall_h[:], ctxall[:])
                    with (
                        tc.tile_pool(name="lo", bufs=2) as lopool,
                        tc.tile_pool(name="bo", bufs=1) as bopool,
                        tc.tile_pool(name="psl", bufs=2, space="PSUM") as psl,
                    ):
                        boutsb = bopool.tile([B, VS], F32, name="boutsb")
                        nc.sync.dma_start(boutsb[:], boutr[:])
                        # chunk pairs with the k-loop interleaved across two
                        # PSUM banks so accumulate chains pipeline on PE
                        for n0 in range(0, nchunks, 2):
                            pair = [n for n in (n0, n0 + 1) if n < nchunks]
                            pls, ws = {}, {}
                            for n in pair:
                                ws[n] = min(CW, VS - n * CW)
                                pls[n] = psl.tile(
                                    [B, CW], F32, name=f"pl{n}", tag=f"pl{n % 2}"
                                )
                            for k in range(NK):
                                for n in pair:
                                    w = ws[n]
                                    nc.tensor.matmul(
                                        pls[n][:, :w],
                                        lhsT=ctxall_h[:, 8 * k : 8 * (k + 1)],
                                        rhs=wotiles[n][0][
                                            :, k * w : (k + 1) * w
                                        ],
                                        start=(k == 0),
                                        stop=(k == NK - 1),
                                    )
                            for n in pair:
                                w = ws[n]
                                vsl = slice(n * CW, n * CW + w)
                                lt = lopool.tile(
                                    [B, CW], F32, name="lt", tag="lt"
                                )
                                nc.vector.tensor_add(
                                    lt[:, :w], pls[n][:, :w], boutsb[:, vsl]
                                )
                                nc.sync.dma_start(logits[:, vsl], lt[:, :w])

    return nc


def _host_prep(inputs):
    f32 = lambda a: np.ascontiguousarray(np.asarray(a, dtype=np.float32))
    seq = np.asarray(inputs["seq"])
    embed = f32(inputs["embed"])
    w1 = f32(inputs["W1"])
    b1 = f32(inputs["b1"])
    w2 = f32(inputs["W2"])
    b2 = f32(inputs["b2"])
    ln_g = f32(inputs["ln_g"])
    ln_b = f32(inputs["ln_b"])
    gw = f32(inputs["gate_W"])
    gb = f32(inputs["gate_b"])
    qw = f32(inputs["q_W"])
    qb = f32(inputs["q_b"])
    wout = f32(inputs["out_W"])
    bout = f32(inputs["out_b"])

    colpack = lambda v: np.ascontiguousarray(
        v.reshape(-1, 128).T.astype(np.float32)
    )  # (Ntiles*128,) -> (128, Ntiles); tile k col = dims [128k, 128k+128)
    cnt = np.minimum(K, T - 1 - np.arange(T)).astype(np.float32)
    cntrec = np.zeros(T, dtype=np.float32)
    cntrec[cnt > 0] = 1.0 / cnt[cnt > 0]

    gw1 = gw[:D, 0]
    gw2 = gw[D:, 0]
    g1g = colpack(gw1 * ln_g)
    g2g = colpack(gw2 * ln_g)
    sgw_in = np.zeros((128, 4 * NK), dtype=np.float32)
    for k in range(NK):
        sgw_in[:, 4 * k] = 1.0
        sgw_in[:, 4 * k + 1] = g1g[:, k]
        sgw_in[:, 4 * k + 2] = g2g[:, k]
    A1 = float(np.dot(gw1, ln_g))
    B1tot = float(np.dot(gw1, ln_b) + gb[0])
    A2 = float(np.dot(gw2, ln_g))
    B2 = float(np.dot(gw2, ln_b))

    cpack = np.zeros((128, 84), dtype=np.float32)
    cpack[:, 0:8] = colpack(b1)
    cpack[:, 8:12] = colpack(b2)
    cpack[:, 12:16] = colpack(ln_g)
    cpack[:, 16:20] = colpack(ln_b)
    cpack[:, 20:24] = colpack(qb)
    cpack[:, 24:40] = sgw_in
    # alpha scaled for tau = alpha*(d/128) + lo
    cpack[:, 40] = np.arange(128, dtype=np.float32) + 1.0
    cpack[:, 41:73] = cntrec.reshape(128, TC)
    cpack[0, 73:81] = [EPS, -A1, B1tot, -A2, B2, 0.0, 0.0, 0.0]
    idsh = np.zeros((128, 256), dtype=np.float32)
    idsh[:, 0:128] = np.eye(128, dtype=np.float32)
    idsh[:, 128:256] = np.eye(128, k=-1, dtype=np.float32)

    f16 = lambda a: np.ascontiguousarray(np.asarray(a, dtype=np.float16))
    base = {
        "embed": f16(embed),
        "w1": f16(w1),
        "w2": f16(w2),
        "qw": f16(qw),
        "cpack": cpack,
        "idsh": idsh,
        "cntrow": np.ascontiguousarray(cntrec.reshape(1, T).astype(np.float16)),
    }
    wout_pad = np.zeros((D, NCORES * VS), dtype=np.float32)
    wout_pad[:, :V] = wout
    bout_pad = np.zeros(NCORES * VS, dtype=np.float32)
    bout_pad[:V] = bout

    nchunks = (VS + CW - 1) // CW
    in_maps = []
    for c in range(NCORES):
        m = dict(base)
        m["seq_idx"] = np.ascontiguousarray(seq[c].reshape(TC, 128).T.astype(np.int32))
        shard = wout_pad[:, c * VS : (c + 1) * VS].astype(np.float16)
        wt = np.zeros((128, nchunks * NK * CW), dtype=np.float16)
        for n in range(nchunks):
            w = min(CW, VS - n * CW)
            blk = shard[:, n * CW : n * CW + w]  # (D, w)
            # [p, k*w + j] = shard[128k + p, n*CW + j]
            wt[:, n * NK * CW : n * NK * CW + NK * w] = (
                blk.reshape(NK, 128, w).transpose(1, 0, 2).reshape(128, NK * w)
            )
        m["wout"] = np.ascontiguousarray(wt)
        m["boutr"] = np.ascontiguousarray(
            np.broadcast_to(bout_pad[c * VS : (c + 1) * VS], (B, VS)).astype(
                np.float32
            )
        )
        in_maps.append(m)
    return in_maps


def get_nc():
    key = (PURE_FP32, DEBUG_HT)
    if key not in _cache:
        _cache[key] = build_bass()
    return _cache[key]


def run_full(inputs, trace=False):
    """Run the kernel; returns (logits_full, BassKernelResults)."""
    nc = get_nc()
    in_maps = _host_prep(inputs)
    res = run_bass_kernel_spmd(
        nc, in_maps, core_ids=list(range(NCORES)), trace=trace
    )
    parts = [res.results[c]["logits"] for c in range(NCORES)]
    logits = np.concatenate(parts, axis=1)[:, :V]
    return logits, res


def kernel(**inputs) -> np.ndarray:
    logits, _ = run_full(inputs, trace=False)
    return logits



# revision 80
# speedup vs baseline: 1.1398x; 1.1398x over previous
"""Trainium2 Bass kernel for nn_LookaheadModel (topk_masking).

Sharding: data-parallel over batch B=8 (one batch element per core) for the
encoder; tiny AllGather of per-batch context vectors; vocab-sharded output
projection (each core computes logits[:, shard]).

v2 layout: activations kept feature-major in the pre-layernorm basis y
(yT tiles: D on partitions, T free).  LN is folded into per-token row
scalars r_t = rsqrt(var+eps), mr_t = mean*r: h = r*(g.*y) - mr*g + b.
Gate / future-phi / score projections are fused into one 3-row matmul per
chunk against host-premultiplied (gate_W .* ln_g) vectors, with row-space
corrections.  All top-k selection and softmax math runs in a column-major
(128 x 32) layout so vector ops use all 128 lanes.  out_W (f16) is fully
prefetched during the encoder so the final projection is PE-bound.

Self-contained: only needs numpy + the system-installed concourse package.
"""

import numpy as np

import bass_rust
import concourse.bass as bass
import concourse.mybir as mybir
from concourse.bass_utils import run_bass_kernel_spmd
from concourse.tile import TileContext

AF = mybir.ActivationFunctionType
ALU = mybir.AluOpType
F32 = mybir.dt.float32
F32R = mybir.dt.float32r
F16 = mybir.dt.float16
BF16 = mybir.dt.bfloat16
I32 = mybir.dt.int32
U8 = mybir.dt.uint8

# ---------------------------------------------------------------------------
# Workaround: this walrus build rejects any instruction carrying more than one
# sync-wait command. Hoist excess waits onto same-engine NOPs (sequential on
# the same engine queue, so semantically identical).
# ---------------------------------------------------------------------------
_MAX_WAITS = 1
_nop_counter = [0]


def _split_waits_in_ordered(nc, ordered):
    for bb_name, insts in ordered.items():
        out = []
        for inst in insts:
            si = inst.sync_info
            waits = list(si.on_wait) if si and si.on_wait else []
            if len(waits) > _MAX_WAITS:
                spill, keep = waits[:-_MAX_WAITS], waits[-_MAX_WAITS:]
                for i in range(0, len(spill), _MAX_WAITS):
                    _nop_counter[0] += 1
                    nop = bass_rust.InstNoOp(name=f"WSPILL-{_nop_counter[0]}")
                    nop.engine = inst.engine
                    nop.sync_info = mybir.SyncInfo(
                        on_wait=list(spill[i : i + _MAX_WAITS]), on_update=[]
                    )
                    nop.bass_nofuse = True
                    nc.register_instruction(nop, overwrite=True)
                    out.append(nop)
                si.on_wait = keep
            out.append(inst)
        if len(out) != len(insts):
            insts[:] = out


_orig_lower = TileContext._lower_ordered_insts
_orig_drain = TileContext._drain_and_barrier


def _lower_with_split(self, ordered):
    _split_waits_in_ordered(self.nc, ordered)
    return _orig_lower(self, ordered)


def _drain_and_barrier_split(self, tick_clock, wait_clock):
    nc = self.nc
    sc = bass_rust.ScopedClock({None: tick_clock.global_clock})
    drain_inst = nc.sync.drain()
    wait_clock.add_sem_waits(drain_inst.ins, sc)
    si = drain_inst.ins.sync_info
    waits = list(si.on_wait or [])
    if len(waits) > _MAX_WAITS:
        si.on_wait = waits[:_MAX_WAITS]
        rest = waits[_MAX_WAITS:]
        for i in range(0, len(rest), _MAX_WAITS):
            nop = nc.sync.nop(nofuse=True, hint=f"drain_wait_spill_{i}")
            nop.ins.sync_info = mybir.SyncInfo(
                on_wait=list(rest[i : i + _MAX_WAITS]), on_update=[]
            )
    nc.all_engine_barrier()
    popped = nc._tile_sem_poison_stack.pop()
    assert popped is self._sem_poison
    nc.clear_and_free_semaphores(list(self.sems.allocated().values()))
    nc.all_engine_barrier()


def _apply_patch():
    TileContext._drain_and_barrier = _drain_and_barrier_split
    TileContext._lower_ordered_insts = _lower_with_split


# ---------------------------------------------------------------------------
# Problem constants
# ---------------------------------------------------------------------------
V, D, SLOTS, K = 50257, 512, 256, 8
B, T = 8, 4096
NCORES = 8
VS = 6283  # vocab shard width per core; 8*6283 = 50264 >= V
NCH = 8  # T chunks of width 512
CW = 512
NK = D // 128  # 4 feature tiles
NF = 2 * D // 128  # 8 hidden tiles
TC = T // 128  # 32 col-layout width
BIG = 1.0e30
EPS = 1e-5

PURE_FP32 = False  # kept for test.py compat (ignored; kernel is f32r/f16)
USE_PBCAST = False  # partition_broadcast unsupported by this walrus codegen
USE_GP_CTXY = False  # Pool engine lacks TensorScalarPtr
DEBUG_HT = False  # adds dbg row dumps (bring-up only)

_cache = {}


def build_bass():
    _apply_patch()
    DT = F16
    FDT = F16
    nc = bass.Bass(trn_type="TRN2", num_devices=NCORES)

    # ---- I/O ----
    embed = nc.dram_tensor("embed", (V, D), F16, kind="ExternalInput")
    seq_idx = nc.dram_tensor("seq_idx", (128, TC), I32, kind="ExternalInput")
    w1 = nc.dram_tensor("w1", (D, 2 * D), F16, kind="ExternalInput")
    w2 = nc.dram_tensor("w2", (2 * D, D), F16, kind="ExternalInput")
    qw = nc.dram_tensor("qw", (D, D), F16, kind="ExternalInput")
    # packed small constants; see cpk layout at the load site
    cpack_in = nc.dram_tensor("cpack", (128, 84), F32, kind="ExternalInput")
    idsh_in = nc.dram_tensor("idsh", (128, 256), F32, kind="ExternalInput")
    NCHUNKS = (VS + CW - 1) // CW
    wout = nc.dram_tensor("wout", (128, NCHUNKS * NK * CW), FDT, kind="ExternalInput")
    boutr = nc.dram_tensor("boutr", (B, VS), F32, kind="ExternalInput")

    logits = nc.dram_tensor("logits", (B, VS), F32, kind="ExternalOutput")
    dbg = nc.dram_tensor("dbg", (8, T), F32, kind="ExternalOutput")

    cntrow_in = nc.dram_tensor("cntrow", (1, T), F16, kind="ExternalInput")
    rows_d = nc.dram_tensor("rows_d", (3, T), F32, kind="Internal")
    zrow_d = nc.dram_tensor("zrow_d", (1, T), F32, kind="Internal")
    wrow_d = nc.dram_tensor("wrow_d", (1, T), F32, kind="Internal")
    wrowh_d = nc.dram_tensor("wrowh_d", (1, T), BF16, kind="Internal")
    cc_din = nc.dram_tensor("cc_din", (128, 1), F32, kind="Internal")
    cc_dout = nc.dram_tensor(
        "cc_dout", (128 * NCORES, 1), F32, kind="Internal", addr_space="Shared"
    )
    cc_in = nc.dram_tensor("cc_in", (128, NK), F32, kind="Internal")
    cc_out = nc.dram_tensor(
        "cc_out", (128 * NCORES, NK), F32, kind="Internal", addr_space="Shared"
    )

    def col2row(drow, ctile):
        # SBUF col tile (128, TC) -> DRAM row (1, T), t = TC*p + c
        return drow[:].rearrange("o (p c) -> (o p) c", p=128), ctile[:]

    with TileContext(nc) as tc:
        with tc.tile_pool(name="consts", bufs=1) as cpool:
            # ---------------- persistent constants ----------------
            # sidx first: the first gather (critical path) depends only on it
            sidx = cpool.tile([128, TC], I32, name="sidx")
            nc.sync.dma_start(sidx[:], seq_idx[:])
            # all small constants packed into one DMA (cpack (128, 84)):
            # [b1 0:8 | b2 8:12 | g4 12:16 | b4 16:20 | qb4 20:24 | sgw 24:40 |
            #  alpha 40 | cntrec 41:73 | row0 73:81 = scal]
            cpk = cpool.tile([128, 84], F32, name="cpk")
            nc.sync.dma_start(cpk[:], cpack_in[:])
            b1t = cpk[:, 0:8]
            b2t = cpk[:, 8:12]
            g4c = cpk[:, 12:16]
            b4c = cpk[:, 16:20]
            qb4c = cpk[:, 20:24]
            cntc = cpk[:, 41:73]
            alphac = cpk[:, 40:41]
            scin = cpk[0:1, 73:81]
            eps_ap = cpk[0:1, 73:74]
            idsh = cpool.tile([128, 256], F32, name="idsh")
            nc.sync.dma_start(idsh[:], idsh_in[:])
            ident = idsh[:, 0:128]
            sht = idsh[:, 128:256]
            # fused per-k-tile lhsT: [g1g, g2g, qg(later), ones] per k
            sgw = cpool.tile([128, 4 * NK], DT, name="sgw")
            nc.vector.tensor_copy(sgw[:], cpk[:, 24:40])
            onescol = cpool.tile([128, 1], F32, name="onescol")
            nc.vector.memset(onescol[:], 1.0)
            identr = cpool.tile([128, 128], DT, name="identr")
            nc.vector.tensor_copy(identr[:], ident)
            onescol_r = cpool.tile([128, 1], F32R, name="onescol_r")
            nc.vector.tensor_copy(onescol_r[:], onescol[:])
            ones1x128 = cpool.tile([1, 128], F32, name="ones1x128")
            nc.vector.memset(ones1x128[:], 1.0)
            ones1x128h = cpool.tile([1, 128], BF16, name="ones1x128h")
            nc.vector.memset(ones1x128h[:], 1.0)
            strip = cpool.tile([1, 64], F32, name="strip")
            nc.vector.memset(strip[0:1, 40:41], BIG)
            nc.vector.memset(strip[0:1, 41:42], -BIG)
            q4 = cpool.tile([128, NK], F32, name="q4")
            hl = cpool.tile([128, NK], DT, name="hl")
            ctxY = cpool.tile([128, NK], F32, name="ctxY")
            ctx4 = cpool.tile([128, NK], F32, name="ctx4")
            ctxall = cpool.tile([128, 4 * NCORES], F32, name="ctxall")

            nchunks = (VS + CW - 1) // CW
            NWO_BUF = 13  # all out_W chunks resident (full prefetch)
            with tc.tile_pool(name="wo", bufs=NWO_BUF) as wopool:
                # out_W streamed in chunk-sized single DMAs (f16); the 4 k-tile
                # blocks of a chunk land side by side in one [128, 4w] tile
                wotiles = []

                def load_wochunk(n, eng=None):
                    w = min(CW, VS - n * CW)
                    wt = wopool.tile([128, NK * CW], FDT, name=f"wo{n}", tag="wo")
                    (eng or nc.sync).dma_start(
                        wt[:, : NK * w],
                        wout[:, n * NK * CW : n * NK * CW + NK * w],
                    )
                    wotiles.append((wt, w))

                with tc.tile_pool(name="yT", bufs=1) as hpool:
                    yT = [hpool.tile([128, T], DT, name=f"yT{k}") for k in range(NK)]
                    # row-space pipeline state lives in its own pool, closed
                    # before the projection so its SBUF is reusable
                    _rowcm = tc.tile_pool(name="rows", bufs=1)
                    rowpool = _rowcm.__enter__()
                    phirow = rowpool.tile([1, T + 8], F32, name="phirow")
                    nc.vector.memset(phirow[0:1, T : T + 8], 0.0)
                    zrowA = rowpool.tile([1, T], F32, name="zrowA")
                    cntrow = rowpool.tile([1, T], F16, name="cntrow")
                    nc.sync.dma_start(cntrow[:], cntrow_in[:])

                    # ---------------- phase A ----------------
                    with (
                        tc.tile_pool(name="wts", bufs=1) as wpool,
                        tc.tile_pool(name="gat", bufs=4) as gpool,
                        tc.tile_pool(name="x0p", bufs=2) as x0pool,
                        tc.tile_pool(name="ap", bufs=1) as apool,
                        tc.tile_pool(name="sqp", bufs=2) as sqpool,
                        tc.tile_pool(name="stp", bufs=2) as stpool,
                        tc.tile_pool(name="st1", bufs=1) as st1pool,
                        tc.tile_pool(name="pstp", bufs=2, space="PSUM") as pstp,
                        tc.tile_pool(name="psa", bufs=2, space="PSUM") as psa,
                        tc.tile_pool(name="psf", bufs=2, space="PSUM") as psf,
                        tc.tile_pool(name="prow", bufs=1, space="PSUM") as prows,
                        tc.tile_pool(name="paux", bufs=1, space="PSUM") as paux,
                    ):
                        # f16 weights: DMA straight into matmul operand tiles
                        w1t, w2t, qwt = [], [], []
                        for k in range(NK):
                            wr = wpool.tile([128, 2 * D], DT, name=f"w1r{k}")
                            nc.sync.dma_start(wr[:], w1[128 * k : 128 * (k + 1), :])
                            w1t.append(wr)
                        for k in range(NF):
                            wr = wpool.tile([128, D], DT, name=f"w2r{k}")
                            nc.sync.dma_start(wr[:], w2[128 * k : 128 * (k + 1), :])
                            w2t.append(wr)
                        for k in range(NK):
                            wr = wpool.tile([128, D], DT, name=f"qwr{k}")
                            nc.sync.dma_start(wr[:], qw[128 * k : 128 * (k + 1), :])
                            qwt.append(wr)
                        # warm the collective stream early: a dummy
                        # AllGather absorbs the fixed ~11.5us trigger-to-start
                        # cost during the encoder instead of the real CC
                        nc.gpsimd.collective_compute(
                            "AllGather",
                            ALU.bypass,
                            replica_groups=[list(range(NCORES))],
                            ins=[cc_din[:]],
                            outs=[cc_dout[:]],
                        )
                        # out_W chunk 0 now; the rest stream 2-per-chunk from
                        # inside the loop so no queue jams at startup
                        load_wochunk(0, nc.sync)
                        wo_next = [1]

                        aux = paux.tile([128, CW], F32, name="aux")

                        order = [7] + list(range(7))
                        for idx, ch in enumerate(order):
                            sl = slice(ch * CW, (ch + 1) * CW)
                            x0 = [
                                x0pool.tile([128, CW], DT, name=f"x0_{k}", tag=f"x0_{k}")
                                for k in range(NK)
                            ]
                            for blk in range(4):
                                tb = 4 * ch + blk
                                g = gpool.tile([128, D], DT, name="g", tag="g")
                                nc.gpsimd.indirect_dma_start(
                                    out=g[:],
                                    out_offset=None,
                                    in_=embed[:],
                                    in_offset=bass.IndirectOffsetOnAxis(
                                        ap=sidx[:, tb : tb + 1], axis=0
                                    ),
                                )
                                tp = pstp.tile([128, D], DT, tag="tp")
                                for k in range(NK):
                                    nc.tensor.transpose(
                                        tp[:, 128 * k : 128 * (k + 1)],
                                        g[:, 128 * k : 128 * (k + 1)],
                                        identr,
                                    )
                                for k in range(NK):
                                    if k % 2 == 0:
                                        nc.vector.tensor_copy(
                                            x0[k][:, 128 * blk : 128 * (blk + 1)],
                                            tp[:, 128 * k : 128 * (k + 1)],
                                        )
                                    else:
                                        nc.scalar.activation(
                                            x0[k][:, 128 * blk : 128 * (blk + 1)],
                                            tp[:, 128 * k : 128 * (k + 1)],
                                            AF.Copy,
                                        )
                            # layer 1 + relu
                            af = [
                                apool.tile([128, CW], DT, name=f"af{m}", tag=f"af{m}")
                                for m in range(NF)
                            ]
                            for m in range(NF):
                                ps = psa.tile([128, CW], F32, tag="psa")
                                for k in range(NK):
                                    nc.tensor.matmul(
                                        ps[:],
                                        lhsT=w1t[k][:, 128 * m : 128 * (m + 1)],
                                        rhs=x0[k][:],
                                        start=(k == 0),
                                        stop=(k == NK - 1),
                                    )
                                nc.scalar.activation(
                                    af[m][:], ps[:], AF.Relu, bias=b1t[:, m : m + 1]
                                )
                            # layer 2 + bias + residual -> yT directly
                            for m in range(NK):
                                ps = psf.tile([128, CW], F32, tag="psf")
                                for k in range(NF):
                                    nc.tensor.matmul(
                                        ps[:],
                                        lhsT=w2t[k][:, 128 * m : 128 * (m + 1)],
                                        rhs=af[k][:],
                                        start=(k == 0),
                                        stop=(k == NF - 1),
                                    )
                                nc.vector.scalar_tensor_tensor(
                                    out=yT[m][:, sl],
                                    in0=ps[:],
                                    scalar=b2t[:, m : m + 1],
                                    in1=x0[m][:],
                                    op0=ALU.add,
                                    op1=ALU.add,
                                )
                            # fused rows matmul: [sum(y), s1, s2, sq] in one
                            # 4-row group per k-tile (sq row needs qg — garbage
                            # for the first chunk (7), patched after q below)
                            pr = prows.tile([128, CW], F32, tag="rows")
                            for k in range(NK):
                                nc.tensor.matmul(
                                    pr[0:4, :],
                                    lhsT=sgw[:, 4 * k : 4 * k + 4],
                                    rhs=yT[k][:, sl],
                                    start=(k == 0),
                                    stop=(k == NK - 1),
                                )
                            nrows = 3 if idx == 0 else 4
                            stg = stpool.tile([4, CW], F32, name="stg", tag="stg")
                            nc.vector.tensor_copy(stg[0:nrows, :], pr[0:nrows, :])
                            if idx > 0:
                                nc.sync.dma_start(rows_d[0:1, sl], stg[3:4, :])
                            # s1/s2 rows to partition 0 (engines can't read
                            # SBUF at partition offsets 1-2; DMAs can)
                            s12 = stpool.tile([1, 2 * CW], F32, name="s12", tag="s12")
                            nc.sync.dma_start(s12[0:1, :], stg[1:3, :])
                            for m in range(NK):
                                sq = sqpool.tile([128, CW], F32R, name="sq", tag="sq")
                                nc.scalar.activation(sq[:], yT[m][:, sl], AF.Square)
                                nc.tensor.matmul(
                                    aux[0:1, :],
                                    lhsT=onescol_r[:],
                                    rhs=sq[:],
                                    start=(m == 0),
                                    stop=(m == NK - 1),
                                )
                            # row chain: r = rsqrt(var+eps) via Ln/Exp; mr = m*r
                            st = st1pool.tile([1, 9 * CW + 32], F32, name="st", tag="st")
                            mrow = st[0:1, 0:CW]
                            ex2 = st[0:1, CW : 2 * CW]
                            t1r = st[0:1, 2 * CW : 3 * CW]
                            r_sl = st[0:1, 3 * CW : 4 * CW]
                            mr_sl = st[0:1, 4 * CW : 5 * CW]
                            nc.vector.tensor_scalar(
                                out=mrow, in0=pr[0:1, :], scalar1=1.0 / D,
                                scalar2=None, op0=ALU.mult,
                            )
                            nc.vector.tensor_scalar(
                                out=ex2, in0=aux[0:1, :], scalar1=1.0 / D,
                                scalar2=None, op0=ALU.mult,
                            )
                            nc.vector.tensor_mul(t1r, mrow, mrow)
                            nc.vector.tensor_sub(ex2, ex2, t1r)
                            nc.scalar.activation(t1r, ex2, AF.Ln, bias=eps_ap)
                            nc.scalar.activation(r_sl, t1r, AF.Exp, scale=-0.5)
                            nc.vector.tensor_mul(mr_sl, mrow, r_sl)
                            nc.sync.dma_start(rows_d[1:2, sl], r_sl)
                            nc.sync.dma_start(rows_d[2:3, sl], mr_sl)
                            for _ in range(2):
                                if wo_next[0] < nchunks:
                                    load_wochunk(wo_next[0], nc.sync)
                                    wo_next[0] += 1

                            # row-space gate pipeline: phi and the z base for
                            # this chunk; finalize z of the chunk whose future
                            # window is now complete
                            ptmp = st[0:1, 5 * CW : 6 * CW]
                            nc.vector.tensor_mul(ptmp, r_sl, s12[0:1, CW : 2 * CW])
                            nc.vector.scalar_tensor_tensor(
                                out=ptmp, in0=mr_sl, scalar=cpk[0:1, 76:77],
                                in1=ptmp, op0=ALU.mult, op1=ALU.add,
                            )
                            nc.vector.tensor_scalar(
                                out=phirow[0:1, sl], in0=ptmp,
                                scalar1=cpk[0:1, 77:78], scalar2=None, op0=ALU.add,
                            )
                            nc.vector.tensor_mul(ptmp, r_sl, s12[0:1, 0:CW])
                            nc.vector.scalar_tensor_tensor(
                                out=ptmp, in0=mr_sl, scalar=cpk[0:1, 74:75],
                                in1=ptmp, op0=ALU.mult, op1=ALU.add,
                            )
                            nc.vector.tensor_scalar(
                                out=zrowA[0:1, sl], in0=ptmp,
                                scalar1=cpk[0:1, 75:76], scalar2=None, op0=ALU.add,
                            )

                            def fin_z(c):
                                s = c * CW
                                arow = st[0:1, 6 * CW : 6 * CW + 518]
                                brow = st[0:1, 7 * CW + 8 : 7 * CW + 524]
                                wrow = st[0:1, 8 * CW + 16 : 8 * CW + 528]
                                nc.vector.tensor_add(
                                    arow,
                                    phirow[0:1, s + 1 : s + 519],
                                    phirow[0:1, s + 2 : s + 520],
                                )
                                nc.vector.tensor_add(
                                    brow,
                                    st[0:1, 6 * CW : 6 * CW + 516],
                                    st[0:1, 6 * CW + 2 : 6 * CW + 518],
                                )
                                nc.vector.tensor_add(
                                    wrow,
                                    st[0:1, 7 * CW + 8 : 7 * CW + 520],
                                    st[0:1, 7 * CW + 12 : 7 * CW + 524],
                                )
                                nc.vector.tensor_mul(
                                    wrow, wrow, cntrow[0:1, s : s + CW]
                                )
                                nc.vector.tensor_add(
                                    zrowA[0:1, s : s + CW],
                                    zrowA[0:1, s : s + CW],
                                    wrow,
                                )
                                # running bounds for the bisection
                                zmn = st[0:1, 6 * CW : 6 * CW + 1]
                                zmx = st[0:1, 6 * CW + 1 : 6 * CW + 2]
                                nc.vector.tensor_reduce(
                                    out=zmn, in_=zrowA[0:1, s : s + CW],
                                    axis=mybir.AxisListType.X, op=ALU.min,
                                )
                                nc.vector.reduce_max(
                                    out=zmx, in_=zrowA[0:1, s : s + CW],
                                    axis=mybir.AxisListType.X,
                                )
                                nc.vector.tensor_tensor(
                                    out=strip[0:1, 40:41], in0=strip[0:1, 40:41],
                                    in1=zmn, op=ALU.min,
                                )
                                nc.vector.tensor_tensor(
                                    out=strip[0:1, 41:42], in0=strip[0:1, 41:42],
                                    in1=zmx, op=ALU.max,
                                )

                            if idx == 0:
                                fin_z(7)
                            elif idx >= 2:
                                fin_z(ch - 1)
                            if idx == 7:
                                fin_z(6)

                            if idx == 0:
                                # ---- q vector from the last token (chunk 7) ----
                                # bcast last-token r/mr (partition 0 slices)
                                nc.tensor.matmul(
                                    aux[:, 8:9], lhsT=ones1x128[:],
                                    rhs=r_sl[0:1, CW - 1 : CW], start=True, stop=True,
                                )
                                nc.tensor.matmul(
                                    aux[:, 9:10], lhsT=ones1x128[:],
                                    rhs=mr_sl[0:1, CW - 1 : CW], start=True, stop=True,
                                )
                                rlB = cpool.tile([128, 2], F32, name="rlB")
                                nc.vector.tensor_copy(rlB[:], aux[:, 8:10])
                                ylast = cpool.tile([128, NK], F32, name="ylast")
                                for k in range(NK):
                                    nc.vector.tensor_copy(
                                        ylast[:, k : k + 1],
                                        yT[k][:, T - 1 : T],
                                    )
                                # hl = (ylast*r - mr) * g + b
                                nc.vector.tensor_scalar(
                                    out=ylast[:], in0=ylast[:],
                                    scalar1=rlB[:, 0:1], scalar2=None, op0=ALU.mult,
                                )
                                nc.vector.tensor_scalar(
                                    out=ylast[:], in0=ylast[:],
                                    scalar1=rlB[:, 1:2], scalar2=None,
                                    op0=ALU.subtract,
                                )
                                nc.vector.tensor_mul(ylast[:], ylast[:], g4c[:])
                                nc.vector.tensor_add(hl[:], ylast[:], b4c[:])
                                # q row = hl^T @ qW + qb
                                for k in range(NK):
                                    nc.tensor.matmul(
                                        aux[0:1, :],
                                        lhsT=hl[:, k : k + 1],
                                        rhs=qwt[k][:],
                                        start=(k == 0),
                                        stop=(k == NK - 1),
                                    )
                                qrow = cpool.tile([1, D], F32, name="qrow")
                                nc.vector.tensor_copy(qrow[:], aux[0:1, :])
                                # transpose q row -> q4 cols; add qb in col form
                                for k in range(NK):
                                    nc.tensor.transpose(
                                        aux[:, 16 + k : 17 + k],
                                        qrow[0:1, 128 * k : 128 * (k + 1)],
                                        ident[0:1, 0:1],
                                    )
                                nc.vector.tensor_add(
                                    q4[:], aux[:, 16 : 16 + NK], qb4c[:]
                                )
                                # qg into sgw cols 4k+3
                                for k in range(NK):
                                    nc.vector.tensor_mul(
                                        sgw[:, 4 * k + 3 : 4 * k + 4],
                                        q4[:, k : k + 1],
                                        g4c[:, k : k + 1],
                                    )
                                # Aq = sum(q*g), Bq = sum(q*b)
                                qgb = cpool.tile([128, 2 * NK], F32, name="qgb")
                                nc.vector.tensor_mul(qgb[:, 0:NK], q4[:], g4c[:])
                                nc.vector.tensor_mul(qgb[:, NK : 2 * NK], q4[:], b4c[:])
                                nc.tensor.matmul(
                                    aux[0:1, 32 : 32 + 2 * NK],
                                    lhsT=onescol[:],
                                    rhs=qgb[:],
                                    start=True,
                                    stop=True,
                                )
                                # strip[0,0]=Aq, strip[0,1]=Bq
                                nc.vector.tensor_reduce(
                                    out=strip[0:1, 0:1],
                                    in_=aux[0:1, 32 : 32 + NK],
                                    axis=mybir.AxisListType.X,
                                    op=ALU.add,
                                )
                                nc.vector.tensor_reduce(
                                    out=strip[0:1, 1:2],
                                    in_=aux[0:1, 32 + NK : 32 + 2 * NK],
                                    axis=mybir.AxisListType.X,
                                    op=ALU.add,
                                )
                                # deferred sq row for chunk 7 (q now known)
                                sl7 = slice(7 * CW, 8 * CW)
                                prd = prows.tile([128, CW], F32, tag="rows")
                                for k in range(NK):
                                    nc.tensor.matmul(
                                        prd[0:1, :],
                                        lhsT=sgw[:, 4 * k + 3 : 4 * k + 4],
                                        rhs=yT[k][:, sl7],
                                        start=(k == 0),
                                        stop=(k == NK - 1),
                                    )
                                stg7 = stpool.tile(
                                    [4, CW], F32, name="stg", tag="stg"
                                )
                                nc.vector.tensor_copy(stg7[0:1, :], prd[0:1, :])
                                nc.sync.dma_start(rows_d[0:1, sl7], stg7[0:1, :])

                    # ---------------- phase B: col-space selection ----------------
                    with (
                        tc.tile_pool(name="colp", bufs=1) as colp,
                        tc.tile_pool(name="bigp", bufs=1) as bigp,
                        tc.tile_pool(name="psm", bufs=1, space="PSUM") as psm,
                        tc.tile_pool(name="pwd", bufs=2, space="PSUM") as pwd,
                    ):
                        TH = T // 2
                        sm = psm.tile([128, CW], F32, name="sm")
                        zB = bigp.tile([128, T], F32, name="zB")
                        scr = bigp.tile([128, TH], F16, name="scr")
                        scrB = bigp.tile([128, TH], F16, name="scrB")
                        wB = bigp.tile([128, T], BF16, name="wB")
                        scrh = bigp.tile([128, T], F16, name="scrh")

                        # sq/r/mr rows -> col layout in ONE DMA; z comes from
                        # the row-space pipeline (zrowA) built during phase A
                        colpk3 = colp.tile([128, 3 * TC], F32, name="colpk3")
                        nc.sync.dma_start(
                            colpk3[:].rearrange("p (i c) -> p i c", i=3),
                            rows_d[:, :].rearrange("i (p c) -> p i c", p=128),
                        )
                        sqc = colpk3[:, 0 * TC : 1 * TC]
                        rc = colpk3[:, 1 * TC : 2 * TC]
                        mrc = colpk3[:, 2 * TC : 3 * TC]
                        sA = colp.tile([128, 40], F32, name="sA")
                        zc = colp.tile([128, TC], F32, name="zc")
                        uc = colp.tile([128, TC], F32, name="uc")
                        tc_ = colp.tile([128, TC], F32, name="tc_")
                        mq = colp.tile([128, TC], F32, name="mq")
                        gtv = colp.tile([128, TC], F32, name="gtv")
                        selc = colp.tile([128, TC], F32, name="selc")
                        ec = colp.tile([128, TC], F32, name="ec")
                        wcol = colp.tile([128, TC], BF16, name="wcol")
                        mask_u8 = colp.tile([128, TC], U8, name="mask_u8")
                        coltmp = colp.tile([128, 16], F32, name="coltmp")
                        zrow_sb = colp.tile([1, T], F32, name="zrow_sb")
                        wrow_sb = colp.tile([1, T], BF16, name="wrow_sb")
                        rsc = colp.tile([1, 256], F32, name="rsc")

                        def bcast(src11, dst_col):
                            # (1,1) -> (128,1) via PE
                            p = sm[:, 12:13]
                            nc.tensor.matmul(
                                p, lhsT=ones1x128[:], rhs=src11, start=True, stop=True
                            )
                            nc.vector.tensor_copy(dst_col, p)

                        # z row -> col tile via DRAM bounce (off critical path:
                        # zc is only needed after the bisection rounds)
                        nc.sync.dma_start(zrow_d[0:1, :], zrowA[0:1, :])
                        nc.sync.dma_start(
                            zc[:], zrow_d[:].rearrange("o (p c) -> (o p) c", p=128)
                        )
                        # replicate z across partitions straight from zrowA
                        for chx in range(NCH):
                            slx = slice(chx * CW, (chx + 1) * CW)
                            pb = pwd.tile([128, CW], F32, tag="pb")
                            nc.tensor.matmul(
                                pb[:], lhsT=ones1x128[:],
                                rhs=zrowA[0:1, slx], start=True, stop=True,
                            )
                            if chx % 2 == 0:
                                nc.vector.tensor_copy(zB[:, slx], pb[:])
                            else:
                                nc.scalar.activation(zB[:, slx], pb[:], AF.Copy)

                        # lo/hi bounds: accumulated in row space during phase A
                        lo0 = strip[0:1, 40:41]
                        hi0 = strip[0:1, 41:42]

                        N_ROUNDS = 3
                        lo_cur, hi_cur = lo0, hi0
                        si = 6
                        tau_col = coltmp[:, 6:7]
                        dB = coltmp[:, 7:8]
                        loB = coltmp[:, 8:9]
                        cnt_col = coltmp[:, 9:10]
                        cnt_col2 = coltmp[:, 12:13]
                        sgn_col = coltmp[:, 10:11]
                        for r in range(N_ROUNDS):
                            # pack [dd, lo] adjacently, one bcast matmul for both
                            dd = strip[0:1, si : si + 1]
                            nc.vector.tensor_scalar(
                                out=dd, in0=hi_cur, scalar1=lo_cur, scalar2=1.0 / 128,
                                op0=ALU.subtract, op1=ALU.mult,
                            )
                            nc.scalar.activation(
                                strip[0:1, si + 1 : si + 2], lo_cur, AF.Copy
                            )
                            p2 = sm[:, 14:16]
                            nc.tensor.matmul(
                                p2, lhsT=ones1x128[:], rhs=strip[0:1, si : si + 2],
                                start=True, stop=True,
                            )
                            nc.scalar.activation(dB, p2[:, 0:1], AF.Copy)
                            nc.scalar.activation(loB, p2[:, 1:2], AF.Copy)
                            # tau = alpha * (128*dd) + lo == alpha*d + lo
                            nc.vector.scalar_tensor_tensor(
                                out=tau_col, in0=alphac, scalar=dB, in1=loB,
                                op0=ALU.mult, op1=ALU.add,
                            )
                            ntau_col = sA[:, 20:21]
                            nc.vector.tensor_scalar(
                                out=ntau_col, in0=tau_col, scalar1=-1.0,
                                scalar2=None, op0=ALU.mult,
                            )
                            # count split: DVE is_gt on the first half, ACT
                            # Sign on the second half (count = (sgn+TH)/2)
                            sgnB_col = sA[:, 21:22]
                            nc.scalar.activation(
                                scrB[:], zB[:, TH:T], AF.Sign, bias=ntau_col,
                                accum_out=sgnB_col,
                            )
                            nc.vector.scalar_tensor_tensor(
                                out=scr[:],
                                in0=zB[:, 0:TH],
                                scalar=tau_col,
                                in1=zB[:, 0:TH],
                                op0=ALU.is_gt,
                                op1=ALU.bypass,
                                accum_out=cnt_col,
                            )
                            # 2*cntA + sgnB >= 2*SLOTS - TH  <=>  count >= SLOTS
                            cnt2x = sA[:, 22:23]
                            nc.vector.scalar_tensor_tensor(
                                out=cnt2x, in0=cnt_col, scalar=2.0, in1=sgnB_col,
                                op0=ALU.mult, op1=ALU.add,
                            )
                            nc.vector.tensor_scalar(
                                out=sgn_col, in0=cnt2x,
                                scalar1=float(2 * SLOTS - TH),
                                scalar2=None, op0=ALU.is_ge,
                            )
                            pj = sm[0:1, 0:1]
                            nc.tensor.matmul(
                                pj, lhsT=sgn_col, rhs=onescol[:], start=True, stop=True
                            )
                            # lo_n = lo + pj*dd ; hi_n = lo_n + dd
                            lo_n = strip[0:1, si + 2 : si + 3]
                            nc.vector.scalar_tensor_tensor(
                                out=lo_n, in0=pj, scalar=dd, in1=lo_cur,
                                op0=ALU.mult, op1=ALU.add,
                            )
                            hi_n = strip[0:1, si + 3 : si + 4]
                            nc.vector.tensor_add(hi_n, lo_n, dd)
                            lo_cur, hi_cur = lo_n, hi_n
                            si += 4

                        # v0 = min(z > lo_cur) exactly (col space)
                        loB2 = coltmp[:, 11:12]
                        bcast(lo_cur, loB2)
                        nc.vector.tensor_scalar(
                            out=mask_u8[:], in0=zc[:], scalar1=loB2, scalar2=None,
                            op0=ALU.is_gt,
                        )
                        nc.vector.memset(wcol[:], BIG)
                        nc.vector.copy_predicated(wcol[:], mask_u8[:], zc[:])
                        wmin_c = coltmp[:, 12:13]
                        nc.vector.tensor_reduce(
                            out=wmin_c, in_=wcol[:], axis=mybir.AxisListType.X, op=ALU.min
                        )
                        nc.tensor.transpose(sm[0:1, 128:256], wmin_c, ident[:])
                        v0 = strip[0:1, si : si + 1]
                        nc.vector.tensor_reduce(
                            out=v0, in_=sm[0:1, 128:256], axis=mybir.AxisListType.X,
                            op=ALU.min,
                        )
                        # exactly one token sits in the final bisection
                        # window (window << min z-gap), so sel = (z>v0) + (z==v0)
                        vB = coltmp[:, 13:14]
                        bcast(v0, vB)
                        nc.vector.scalar_tensor_tensor(
                            out=gtv[:], in0=zc[:], scalar=vB, in1=zc[:],
                            op0=ALU.is_gt, op1=ALU.bypass,
                        )
                        nc.vector.tensor_scalar(
                            out=mq[:], in0=zc[:], scalar1=vB, scalar2=None,
                            op0=ALU.is_equal,
                        )
                        nc.vector.tensor_add(selc[:], gtv[:], mq[:])

                        # masked softmax over scores (col space)
                        # s = r*sq - Aq*mr + Bq
                        BqB = coltmp[:, 4:5]
                        nAqB = coltmp[:, 5:6]
                        negaq = strip[0:1, 2:3]
                        nc.vector.tensor_scalar(
                            out=negaq, in0=strip[0:1, 0:1], scalar1=-1.0, scalar2=None,
                            op0=ALU.mult,
                        )
                        nc.tensor.matmul(
                            sm[:, 14:16], lhsT=ones1x128[:],
                            rhs=strip[0:1, 1:3], start=True, stop=True,
                        )
                        nc.vector.tensor_copy(coltmp[:, 4:6], sm[:, 14:16])
                        nc.vector.tensor_mul(tc_[:], rc[:], sqc[:])
                        nc.vector.scalar_tensor_tensor(
                            out=tc_[:], in0=mrc[:], scalar=nAqB, in1=tc_[:],
                            op0=ALU.mult, op1=ALU.add,
                        )
                        nc.vector.tensor_scalar(
                            out=tc_[:], in0=tc_[:], scalar1=BqB, scalar2=None, op0=ALU.add
                        )
                        # masked = s + BIG*(sel-1); (sel-1) FIRST to avoid 1e30+s
                        nc.vector.tensor_scalar(
                            out=uc[:], in0=selc[:], scalar1=-1.0, scalar2=None,
                            op0=ALU.add,
                        )
                        nc.vector.scalar_tensor_tensor(
                            out=uc[:], in0=uc[:], scalar=BIG, in1=tc_[:],
                            op0=ALU.mult, op1=ALU.add,
                        )
                        smx_c = coltmp[:, 6:7]
                        nc.vector.reduce_max(
                            out=smx_c, in_=uc[:], axis=mybir.AxisListType.X
                        )
                        nc.tensor.transpose(sm[0:1, 128:256], smx_c, ident[:])
                        smax = strip[0:1, si + 3 : si + 4]
                        nc.vector.reduce_max(
                            out=smax, in_=sm[0:1, 128:256], axis=mybir.AxisListType.X
                        )
                        nsmax = strip[0:1, si + 4 : si + 5]
                        nc.vector.tensor_scalar(
                            out=nsmax, in0=smax, scalar1=-1.0, scalar2=None, op0=ALU.mult
                        )
                        nsB = coltmp[:, 7:8]
                        bcast(nsmax, nsB)
                        zs_col = coltmp[:, 8:9]
                        nc.scalar.activation(
                            ec[:], uc[:], AF.Exp, bias=nsB, accum_out=zs_col
                        )
                        pz = sm[0:1, 2:3]
                        nc.tensor.matmul(
                            pz, lhsT=zs_col, rhs=onescol[:], start=True, stop=True
                        )
                        rz = strip[0:1, si + 5 : si + 6]
                        nc.vector.reciprocal(out=rz, in_=pz)
                        if DEBUG_HT:
                            nc.sync.dma_start(*col2row(dbg[0:1, :], zc))
                            nc.sync.dma_start(*col2row(dbg[1:2, :], selc))
                            nc.sync.dma_start(*col2row(dbg[2:3, :], tc_))  # s
                            nc.sync.dma_start(*col2row(dbg[3:4, :], ec))
                            nc.sync.dma_start(*col2row(dbg[4:5, :], selc))
                        # w = e*r (unnormalized); S2u = sum(e*mr)
                        nc.vector.tensor_mul(wcol[:], ec[:], rc[:])
                        nc.vector.tensor_mul(tc_[:], ec[:], mrc[:])
                        s2p = coltmp[:, 9:10]
                        nc.vector.tensor_reduce(
                            out=s2p, in_=tc_[:], axis=mybir.AxisListType.X, op=ALU.add
                        )
                        ps2u = sm[0:1, 3:4]
                        nc.tensor.matmul(
                            ps2u, lhsT=s2p, rhs=onescol[:], start=True, stop=True
                        )

                        # w col -> row -> replicate into wB (bf16)
                        nc.sync.dma_start(*col2row(wrowh_d, wcol))
                        nc.sync.dma_start(wrow_sb[:], wrowh_d[:])
                        if USE_PBCAST:
                            nc.gpsimd.partition_broadcast(wB[:], wrow_sb[0:1, :])
                        else:
                            for chx in range(NCH):
                                slx = slice(chx * CW, (chx + 1) * CW)
                                pb = pwd.tile([128, CW], F32, tag="pb")
                                nc.tensor.matmul(
                                    pb[:], lhsT=ones1x128h[:],
                                    rhs=wrow_sb[0:1, slx], start=True, stop=True,
                                )
                                if chx % 2 == 0:
                                    nc.vector.tensor_copy(wB[:, slx], pb[:])
                                else:
                                    nc.scalar.activation(wB[:, slx], pb[:], AF.Copy)
                        # ctxY[:, k] = sum_t w_t * y_k[:, t]; token range split
                        # across DVE and GpSimd, combined at the end
                        for k in range(NK):
                            nc.vector.scalar_tensor_tensor(
                                out=scrh[:],
                                in0=yT[k][:],
                                scalar=1.0,
                                in1=wB[:],
                                op0=ALU.mult,
                                op1=ALU.mult,
                                accum_out=ctxY[:, k : k + 1],
                            )
                        # ctx = g*ctxY*rz - (S2u*rz)*g + b
                        rzB = coltmp[:, 10:11]
                        s2rz = strip[0:1, si + 6 : si + 7]
                        nc.vector.tensor_mul(s2rz, ps2u, rz)
                        s2rzB = coltmp[:, 11:12]
                        nc.tensor.matmul(
                            sm[:, 14:16], lhsT=ones1x128[:],
                            rhs=strip[0:1, si + 5 : si + 7], start=True, stop=True,
                        )
                        nc.vector.tensor_copy(coltmp[:, 10:12], sm[:, 14:16])
                        nc.vector.tensor_scalar(
                            out=ctxY[:], in0=ctxY[:], scalar1=rzB, scalar2=None,
                            op0=ALU.mult,
                        )
                        nc.vector.tensor_mul(ctxY[:], ctxY[:], g4c[:])
                        # u = s2rz*g - b ; ctx4 = ctxY - u
                        nc.vector.scalar_tensor_tensor(
                            out=ctx4[:], in0=g4c[:], scalar=s2rzB, in1=b4c[:],
                            op0=ALU.mult, op1=ALU.subtract,
                        )
                        nc.vector.tensor_sub(ctx4[:], ctxY[:], ctx4[:])

                        nc.sync.dma_start(cc_in[:], ctx4[:])
                        nc.gpsimd.collective_compute(
                            "AllGather",
                            ALU.bypass,
                            replica_groups=[list(range(NCORES))],
                            ins=[cc_in[:]],
                            outs=[cc_out[:]],
                        )

                        # PE p-state warmup: a post-CC token DMA gates dummy
                        # matmuls so the PE clock ramps while ctxall lands
                        nc.gpsimd.dma_start(
                            out=yT[0][:, 0:1], in_=cc_out[0:128, 0:1]
                        )
                        for _ in range(14):
                            wm = pwd.tile([128, CW], F32, tag="pb")
                            nc.tensor.matmul(
                                wm[:], lhsT=identr[:], rhs=yT[0][:, 0:CW],
                                start=True, stop=True,
                            )

                    # phase-B + row pools closed; SBUF free for the projection
                    _rowcm.__exit__(None, None, None)

                    # ---------------- output projection ----------------
                    nc.sync.dma_start(
                        ctxall[:].rearrange("p (j b) -> p j b", j=NK),
                        cc_out[:].rearrange("(b p) j -> p j b", p=128),
                    )
                    ctxall_h = cpool.tile([128, 4 * NCORES], F16, name="ctxall_h")
                    nc.vector.tensor_copy(ctxall_h[:], ctxall[:])
                    with (
                        tc.tile_pool(name="lo", bufs=2) as lopool,
                        tc.tile_pool(name="bo", bufs=1) as bopool,
                        tc.tile_pool(name="psl", bufs=2, space="PSUM") as psl,
                    ):
                        boutsb = bopool.tile([B, VS], F32, name="boutsb")
                        nc.sync.dma_start(boutsb[:], boutr[:])
                        # chunk pairs with the k-loop interleaved across two
                        # PSUM banks so accumulate chains pipeline on PE
                        for n0 in range(0, nchunks, 2):
                            pair = [n for n in (n0, n0 + 1) if n < nchunks]
                            pls, ws = {}, {}
                            for n in pair:
                                ws[n] = min(CW, VS - n * CW)
                                pls[n] = psl.tile(
                                    [B, CW], F32, name=f"pl{n}", tag=f"pl{n % 2}"
                                )
                            for k in range(NK):
                                for n in pair:
                                    w = ws[n]
                                    nc.tensor.matmul(
                                        pls[n][:, :w],
                                        lhsT=ctxall_h[:, 8 * k : 8 * (k + 1)],
                                        rhs=wotiles[n][0][
                                            :, k * w : (k + 1) * w
                                        ],
                                        start=(k == 0),
                                        stop=(k == NK - 1),
                                    )
                            for n in pair:
                                w = ws[n]
                                vsl = slice(n * CW, n * CW + w)
                                lt = lopool.tile(
                                    [B, CW], F32, name="lt", tag="lt"
                                )
                                nc.vector.tensor_add(
                                    lt[:, :w], pls[n][:, :w], boutsb[:, vsl]
                                )
                                nc.sync.dma_start(logits[:, vsl], lt[:, :w])

    return nc


def _host_prep(inputs):
    f32 = lambda a: np.ascontiguousarray(np.asarray(a, dtype=np.float32))
    seq = np.asarray(inputs["seq"])
    embed = f32(inputs["embed"])
    w1 = f32(inputs["W1"])
    b1 = f32(inputs["b1"])
    w2 = f32(inputs["W2"])
    b2 = f32(inputs["b2"])
    ln_g = f32(inputs["ln_g"])
    ln_b = f32(inputs["ln_b"])
    gw = f32(inputs["gate_W"])
    gb = f32(inputs["gate_b"])
    qw = f32(inputs["q_W"])
    qb = f32(inputs["q_b"])
    wout = f32(inputs["out_W"])
    bout = f32(inputs["out_b"])

    colpack = lambda v: np.ascontiguousarray(
        v.reshape(-1, 128).T.astype(np.float32)
    )  # (Ntiles*128,) -> (128, Ntiles); tile k col = dims [128k, 128k+128)
    cnt = np.minimum(K, T - 1 - np.arange(T)).astype(np.float32)
    cntrec = np.zeros(T, dtype=np.float32)
    cntrec[cnt > 0] = 1.0 / cnt[cnt > 0]

    gw1 = gw[:D, 0]
    gw2 = gw[D:, 0]
    g1g = colpack(gw1 * ln_g)
    g2g = colpack(gw2 * ln_g)
    sgw_in = np.zeros((128, 4 * NK), dtype=np.float32)
    for k in range(NK):
        sgw_in[:, 4 * k] = 1.0
        sgw_in[:, 4 * k + 1] = g1g[:, k]
        sgw_in[:, 4 * k + 2] = g2g[:, k]
    A1 = float(np.dot(gw1, ln_g))
    B1tot = float(np.dot(gw1, ln_b) + gb[0])
    A2 = float(np.dot(gw2, ln_g))
    B2 = float(np.dot(gw2, ln_b))

    cpack = np.zeros((128, 84), dtype=np.float32)
    cpack[:, 0:8] = colpack(b1)
    cpack[:, 8:12] = colpack(b2)
    cpack[:, 12:16] = colpack(ln_g)
    cpack[:, 16:20] = colpack(ln_b)
    cpack[:, 20:24] = colpack(qb)
    cpack[:, 24:40] = sgw_in
    # alpha scaled for tau = alpha*(d/128) + lo
    cpack[:, 40] = np.arange(128, dtype=np.float32) + 1.0
    cpack[:, 41:73] = cntrec.reshape(128, TC)
    cpack[0, 73:81] = [EPS, -A1, B1tot, -A2, B2, 0.0, 0.0, 0.0]
    idsh = np.zeros((128, 256), dtype=np.float32)
    idsh[:, 0:128] = np.eye(128, dtype=np.float32)
    idsh[:, 128:256] = np.eye(128, k=-1, dtype=np.float32)

    f16 = lambda a: np.ascontiguousarray(np.asarray(a, dtype=np.float16))
    base = {
        "embed": f16(embed),
        "w1": f16(w1),
        "w2": f16(w2),
        "qw": f16(qw),
        "cpack": cpack,
        "idsh": idsh,
        "cntrow": np.ascontiguousarray(cntrec.reshape(1, T).astype(np.float16)),
    }
    wout_pad = np.zeros((D, NCORES * VS), dtype=np.float32)
    wout_pad[:, :V] = wout
    bout_pad = np.zeros(NCORES * VS, dtype=np.float32)
    bout_pad[:V] = bout

    nchunks = (VS + CW - 1) // CW
    in_maps = []
    for c in range(NCORES):
        m = dict(base)
        m["seq_idx"] = np.ascontiguousarray(seq[c].reshape(TC, 128).T.astype(np.int32))
        shard = wout_pad[:, c * VS : (c + 1) * VS].astype(np.float16)
        wt = np.zeros((128, nchunks * NK * CW), dtype=np.float16)
        for n in range(nchunks):
            w = min(CW, VS - n * CW)
            blk = shard[:, n * CW : n * CW + w]  # (D, w)
            # [p, k*w + j] = shard[128k + p, n*CW + j]
            wt[:, n * NK * CW : n * NK * CW + NK * w] = (
                blk.reshape(NK, 128, w).transpose(1, 0, 2).reshape(128, NK * w)
            )
        m["wout"] = np.ascontiguousarray(wt)
        m["boutr"] = np.ascontiguousarray(
            np.broadcast_to(bout_pad[c * VS : (c + 1) * VS], (B, VS)).astype(
                np.float32
            )
        )
        in_maps.append(m)
    return in_maps


def get_nc():
    key = (PURE_FP32, DEBUG_HT)
    if key not in _cache:
        _cache[key] = build_bass()
    return _cache[key]


def run_full(inputs, trace=False):
    """Run the kernel; returns (logits_full, BassKernelResults)."""
    nc = get_nc()
    in_maps = _host_prep(inputs)
    res = run_bass_kernel_spmd(
        nc, in_maps, core_ids=list(range(NCORES)), trace=trace
    )
    parts = [res.results[c]["logits"] for c in range(NCORES)]
    logits = np.concatenate(parts, axis=1)[:, :V]
    return logits, res


def kernel(**inputs) -> np.ndarray:
    logits, _ = run_full(inputs, trace=False)
    return logits

